# revision 11
# baseline (speedup 1.0000x reference)
"""AmpPerLoss distributed Trainium2 kernel.

Data-parallel over the batch axis: 128 samples split across 8 NeuronCores
(16 samples each). Per core, each sample's length-100000 row is laid out as
8 SBUF partitions x 12500 elements, so a core's shard is [128, 12500].

Per-core device work:
  - BCE partials: sum(softplus(p)) via ACT accumulate, sum(p*t) via a fused
    DVE multiply+add-reduce.
  - Window bounds: m = (t > 0.5) / (p > 0); first/last set index via
    subtiled fused multiply+max-reduce against small index ramps; tiny
    fixup algebra; cross-partition (per-sample) combine via DRAM-bounce DMA.
  - Windowed amplitude: 4x tensor_mask_reduce on s and -s with per-row
    [start, end) bounds.
  - Smoothness partials: shifted subtract on sigmoid(p) + ACT Abs
    accumulate (row-boundary pairs finished on host).

The host reduces the 8 cores' partial stats to the final scalar loss
(the "all-reduce" is three scalars).
"""

import sys

if "/opt/trn_rl_repo" not in sys.path:
    sys.path.insert(0, "/opt/trn_rl_repo")

import numpy as np

import concourse.bass as bass
import concourse.bacc as bacc
import concourse.tile as tile
import concourse.mybir as mybir
from concourse.bass_utils import run_bass_kernel_spmd
from concourse import dve_ops

N_CORES = 8
B, L = 128, 100000
BPC = B // N_CORES          # samples per core
CHUNKS = 8                  # partitions per sample
P = BPC * CHUNKS            # 128 partitions
F = L // CHUNKS             # 12500 free elements per row
NB = 10                     # bound-search subtiles per row
SUB = F // NB               # 1250
BIGF = 1.0e30
FMIN = -3.0e38

F32 = mybir.dt.float32
BF16 = mybir.dt.bfloat16
Alu = mybir.AluOpType
Act = mybir.ActivationFunctionType
AX = mybir.AxisListType

# stats column layout
C_WMAX_T, C_WMIN_T, C_WMAX_P, C_WMIN_P = 0, 1, 2, 3
C_SP, C_PT, C_SM = 4, 5, 6
C_SIG0, C_SIGL = 7, 8
NSTAT = 16


def build_nc(n_cores=N_CORES, f=F, nb=NB):
    sub = f // nb
    nc = bacc.Bacc("TRN2", target_bir_lowering=False, debug=False,
                   num_devices=n_cores)

    t_ext = nc.dram_tensor("t", [P, f], F32, kind="ExternalInput")
    p_ext = nc.dram_tensor("p", [P, f], F32, kind="ExternalInput")
    s_ext = nc.dram_tensor("s", [P, f], F32, kind="ExternalInput")
    asc_ext = nc.dram_tensor("asc", [P, sub], F32, kind="ExternalInput")
    desc_ext = nc.dram_tensor("desc", [P, sub], F32, kind="ExternalInput")
    korr_ext = nc.dram_tensor("korr", [P, 4 * nb], F32, kind="ExternalInput")
    sgn_ext = nc.dram_tensor("sgn", [P, 4 * nb], F32, kind="ExternalInput")
    bigs_ext = nc.dram_tensor("bigs", [P, 4 * nb], F32, kind="ExternalInput")
    off_ext = nc.dram_tensor("off", [P, 1], F32, kind="ExternalInput")

    stats_ext = nc.dram_tensor("stats", [P, NSTAT], F32, kind="ExternalOutput")
    b16_ext = nc.dram_tensor("b16", [BPC, 4], F32, kind="ExternalOutput")

    # DRAM bounce buffers for the partition-crossing rearranges
    rowvals_b = nc.dram_tensor("rowvals_b", [P, 4], F32)
    bounds_b = nc.dram_tensor("bounds_b", [BPC, 4], F32)

    with tile.TileContext(nc) as tc:
        with tc.tile_pool(name="big", bufs=1) as big, \
             tc.tile_pool(name="small", bufs=1) as small:
            t_sb = big.tile([P, f], F32, tag="A")
            p_sb = big.tile([P, f], F32, tag="B")
            sig_sb = big.tile([P, f], BF16, tag="C")
            m_sb = big.tile([P, f], BF16, tag="D")

            asc_sb = small.tile([P, sub], F32, tag="asc")
            desc_sb = small.tile([P, sub], F32, tag="desc")
            korr_sb = small.tile([P, 4 * nb], F32, tag="korr")
            sgn_sb = small.tile([P, 4 * nb], F32, tag="sgn")
            bigs_sb = small.tile([P, 4 * nb], F32, tag="bigs")
            off_sb = small.tile([P, 1], F32, tag="off")
            scr = small.tile([P, sub], F32, tag="scr")
            enc = small.tile([P, 4 * nb], F32, tag="enc")
            cm = small.tile([P, 4 * nb], F32, tag="cm")
            dm = small.tile([P, 4 * nb], F32, tag="dm")
            a1 = small.tile([P, 4 * nb], F32, tag="a1")
            t1 = small.tile([P, 4 * nb], F32, tag="t1")
            t2 = small.tile([P, 4 * nb], F32, tag="t2")
            cand = small.tile([P, 4 * nb], F32, tag="cand")
            rowvals = small.tile([P, 4], F32, tag="rowvals")
            comb = small.tile([BPC, CHUNKS, 4], F32, tag="comb")
            b16 = small.tile([BPC, 4], F32, tag="b16")
            bc = small.tile([P, 4], F32, tag="bc")
            rowb = small.tile([P, 4], F32, tag="rowb")
            stats = small.tile([P, NSTAT], F32, tag="stats")
            nc.vector.memset(stats[:, :], 0.0)

            # ---- input DMAs (free-dim chunks so each spans 128 partitions)
            nchunk = 10
            assert f % nchunk == 0
            fc = f // nchunk
            for i in range(nchunk):
                sl = slice(i * fc, (i + 1) * fc)
                nc.sync.dma_start(out=t_sb[:, sl], in_=t_ext.ap()[:, sl])
            for i in range(nchunk):
                sl = slice(i * fc, (i + 1) * fc)
                nc.sync.dma_start(out=p_sb[:, sl], in_=p_ext.ap()[:, sl])
            nc.sync.dma_start(out=asc_sb, in_=asc_ext.ap())
            nc.sync.dma_start(out=desc_sb, in_=desc_ext.ap())
            nc.sync.dma_start(out=korr_sb, in_=korr_ext.ap())
            nc.sync.dma_start(out=sgn_sb, in_=sgn_ext.ap())
            nc.sync.dma_start(out=bigs_sb, in_=bigs_ext.ap())
            nc.sync.dma_start(out=off_sb, in_=off_ext.ap())

            # ---- BCE: sum p*t via the custom-DVE fused multiply+sum-reduce;
            # softplus via softplus(p) = -ln(sigmoid(-p)) on ACT.
            nc.vector._custom_dve(
                dve_ops.TENSOR_TENSOR_REDUCE,
                out=m_sb[:, :], in0=p_sb[:, :], in1=t_sb[:, :],
                s0=0.0, s1=1.0,
                accum_out=stats[:, C_PT:C_PT + 1])

            # ---- sigmoid for smoothness
            nc.scalar.activation(out=sig_sb[:, :], in_=p_sb[:, :],
                                 func=Act.Sigmoid)

            # ---- bound encodes: for mask in {t>0.5, p>0}, per subtile k:
            #   enc_hi[k] = max(m * (j+1)),  enc_lo[k] = max(m * (SUB-j))
            def bound_encodes(src, thr, hi_col0, lo_col0):
                nc.vector.tensor_scalar(out=m_sb[:, :], in0=src[:, :],
                                        scalar1=thr, scalar2=None,
                                        op0=Alu.is_gt)
                for k in range(nb):
                    msl = m_sb[:, k * sub:(k + 1) * sub]
                    nc.vector.tensor_mul(scr[:, :], msl, asc_sb[:, :])
                    nc.vector.tensor_reduce(
                        out=enc[:, hi_col0 + k:hi_col0 + k + 1],
                        in_=scr[:, :], axis=AX.X, op=Alu.max)
                for k in range(nb):
                    msl = m_sb[:, k * sub:(k + 1) * sub]
                    nc.vector.tensor_mul(scr[:, :], msl, desc_sb[:, :])
                    nc.vector.tensor_reduce(
                        out=enc[:, lo_col0 + k:lo_col0 + k + 1],
                        in_=scr[:, :], axis=AX.X, op=Alu.max)

            bound_encodes(t_sb, 0.5, 0, 2 * nb)
            bound_encodes(p_sb, 0.0, nb, 3 * nb)

            # softplus(p) = -ln(sigmoid(-p)); u reuses t's (now dead) slot.
            # stats[C_SP] holds the NEGATED softplus sum; host negates back.
            u_sb = big.tile([P, f], F32, tag="A")
            nc.scalar.activation(out=u_sb[:, :], in_=p_sb[:, :],
                                 func=Act.Sigmoid, scale=-1.0)
            nc.scalar.activation(out=m_sb[:, :], in_=u_sb[:, :],
                                 func=Act.Ln,
                                 accum_out=stats[:, C_SP:C_SP + 1])

            # ---- fixups on enc [P, 4*nb]:
            # hi cols: cand = j_last_in_row = enc-1 + k*SUB   (korr = k*SUB-1)
            # lo cols: cand = j_first_in_row = k*SUB+SUB - enc (korr = k*SUB+SUB)
            # empty subtile (enc==0) -> -+BIGF
            nc.vector.tensor_scalar(out=cm[:, :], in0=enc[:, :], scalar1=0.0,
                                    scalar2=None, op0=Alu.is_gt)
            nc.vector.tensor_scalar(out=dm[:, :], in0=enc[:, :], scalar1=0.0,
                                    scalar2=None, op0=Alu.is_le)
            nc.vector.tensor_mul(a1[:, :], enc[:, :], sgn_sb[:, :])
            nc.vector.tensor_add(a1[:, :], a1[:, :], korr_sb[:, :])
            nc.vector.tensor_mul(t1[:, :], cm[:, :], a1[:, :])
            nc.vector.tensor_mul(t2[:, :], dm[:, :], bigs_sb[:, :])
            nc.vector.tensor_add(cand[:, :], t1[:, :], t2[:, :])

            # ---- per-row reduce over subtiles -> rowvals [P,4]
            # cols: 0 hi_t (max), 1 hi_p (max), 2 lo_t (min), 3 lo_p (min)
            for i, op in ((0, Alu.max), (1, Alu.max), (2, Alu.min), (3, Alu.min)):
                nc.vector.tensor_reduce(
                    out=rowvals[:, i:i + 1],
                    in_=cand[:, i * nb:(i + 1) * nb],
                    axis=AX.X, op=op)
            # row-local -> global position
            nc.vector.tensor_scalar(out=rowvals[:, :], in0=rowvals[:, :],
                                    scalar1=off_sb[:, 0:1], scalar2=None,
                                    op0=Alu.add)

            # ---- per-sample combine via DRAM bounce
            nc.sync.dma_start(out=rowvals_b.ap(), in_=rowvals[:, :])
            nc.sync.dma_start(
                out=comb[:, :, :],
                in_=rowvals_b.ap().rearrange("(b c) k -> b c k", c=CHUNKS))
            combv = comb[:, :, :].rearrange("b c k -> b k c")
            nc.vector.tensor_reduce(out=b16[:, 0:2], in_=combv[:, 0:2, :],
                                    axis=AX.X, op=Alu.max)
            nc.vector.tensor_reduce(out=b16[:, 2:4], in_=combv[:, 2:4, :],
                                    axis=AX.X, op=Alu.min)
            # end = hi + 1 (half-open)
            nc.vector.tensor_scalar(out=b16[:, 0:2], in0=b16[:, 0:2],
                                    scalar1=1.0, scalar2=None, op0=Alu.add)
            nc.sync.dma_start(out=b16_ext.ap(), in_=b16[:, :])

            # ---- broadcast bounds back to rows via DRAM bounce
            nc.sync.dma_start(out=bounds_b.ap(), in_=b16[:, :])
            bap = bounds_b.ap()
            src_bc = bass.AP(tensor=bap.tensor, offset=bap.offset,
                             ap=[[4, BPC], [0, CHUNKS], [1, 4]])
            nc.sync.dma_start(out=bc[:, :], in_=src_bc)
            # row-local bounds: subtract row offset
            nc.vector.tensor_scalar(out=rowb[:, :], in0=bc[:, :],
                                    scalar1=off_sb[:, 0:1], scalar2=None,
                                    op0=Alu.subtract)
            # rowb cols: 0 end_t, 1 end_p, 2 start_t, 3 start_p

            # ---- smoothness: diff of sigmoid within rows, ACT Abs accum
            nc.vector.tensor_sub(m_sb[:, 0:f - 1], sig_sb[:, 1:f],
                                 sig_sb[:, 0:f - 1])
            nc.scalar.activation(out=m_sb[:, 0:f - 1], in_=m_sb[:, 0:f - 1],
                                 func=Act.Abs,
                                 accum_out=stats[:, C_SM:C_SM + 1])
            # boundary sigmoid values for host
            nc.vector.tensor_copy(stats[:, C_SIG0:C_SIG0 + 1], sig_sb[:, 0:1])
            nc.vector.tensor_copy(stats[:, C_SIGL:C_SIGL + 1],
                                  sig_sb[:, f - 1:f])

            # ---- phase 2: windowed amplitude over s
            s_sb = big.tile([P, f], F32, tag="A")
            sneg_sb = big.tile([P, f], F32, tag="B")
            for i in range(nchunk):
                sl = slice(i * fc, (i + 1) * fc)
                nc.sync.dma_start(out=s_sb[:, sl], in_=s_ext.ap()[:, sl])
            nc.vector.tensor_scalar(out=sneg_sb[:, :], in0=s_sb[:, :],
                                    scalar1=-1.0, scalar2=None, op0=Alu.mult)

            # custom TENSOR_MASK_REDUCE: accum = max(s1, max(select(mask,
            # in0, -FLT_MAX) * imm2)); mask is [s0, in1) per row. The min
            # columns hold max(-s) per row; the host negates them back.
            nc.vector._custom_dve(
                dve_ops.TENSOR_MASK_REDUCE,
                out=sig_sb[:, :], in0=s_sb[:, :], in1=rowb[:, 0:1],
                s0=rowb[:, 2:3], s1=FMIN, imm2=1.0,
                accum_out=stats[:, C_WMAX_T:C_WMAX_T + 1])
            nc.vector._custom_dve(
                dve_ops.TENSOR_MASK_REDUCE,
                out=m_sb[:, :], in0=sneg_sb[:, :], in1=rowb[:, 0:1],
                s0=rowb[:, 2:3], s1=FMIN, imm2=1.0,
                accum_out=stats[:, C_WMIN_T:C_WMIN_T + 1])
            nc.vector._custom_dve(
                dve_ops.TENSOR_MASK_REDUCE,
                out=sig_sb[:, :], in0=s_sb[:, :], in1=rowb[:, 1:2],
                s0=rowb[:, 3:4], s1=FMIN, imm2=1.0,
                accum_out=stats[:, C_WMAX_P:C_WMAX_P + 1])
            nc.vector._custom_dve(
                dve_ops.TENSOR_MASK_REDUCE,
                out=m_sb[:, :], in0=sneg_sb[:, :], in1=rowb[:, 1:2],
                s0=rowb[:, 3:4], s1=FMIN, imm2=1.0,
                accum_out=stats[:, C_WMIN_P:C_WMIN_P + 1])

            nc.sync.dma_start(out=stats_ext.ap(), in_=stats[:, :])

    nc.compile()
    return nc


_NC_CACHE = {}


def _get_nc():
    if "nc" not in _NC_CACHE:
        _NC_CACHE["nc"] = build_nc()
    return _NC_CACHE["nc"]


def _make_consts(f=F, nb=NB):
    sub = f // nb
    asc = np.broadcast_to(np.arange(1, sub + 1, dtype=np.float32), (P, sub))
    desc = np.broadcast_to(np.arange(sub, 0, -1, dtype=np.float32), (P, sub))
    k = np.arange(nb, dtype=np.float32)
    korr_hi = k * sub - 1.0
    korr_lo = k * sub + sub
    korr = np.concatenate([korr_hi, korr_hi, korr_lo, korr_lo])
    korr = np.broadcast_to(korr.astype(np.float32), (P, 4 * nb))
    sgn = np.concatenate([np.ones(2 * nb), -np.ones(2 * nb)])
    sgn = np.broadcast_to(sgn.astype(np.float32), (P, 4 * nb))
    bigs = np.concatenate([np.full(2 * nb, -BIGF), np.full(2 * nb, BIGF)])
    bigs = np.broadcast_to(bigs.astype(np.float32), (P, 4 * nb))
    off = ((np.arange(P) % CHUNKS) * f).astype(np.float32).reshape(P, 1)
    return {
        "asc": np.ascontiguousarray(asc),
        "desc": np.ascontiguousarray(desc),
        "korr": np.ascontiguousarray(korr),
        "sgn": np.ascontiguousarray(sgn),
        "bigs": np.ascontiguousarray(bigs),
        "off": off,
    }


def host_combine(results):
    """results: list of dicts with 'stats' [P,16] and 'b16' [BPC,4]."""
    sp_sum = 0.0
    pt_sum = 0.0
    sm_sum = 0.0
    amp_sum = 0.0
    for res in results:
        stats = res["stats"].astype(np.float64)
        b16 = res["b16"].astype(np.float64)
        rows = stats.reshape(BPC, CHUNKS, NSTAT)
        wmax_t = rows[:, :, C_WMAX_T].max(axis=1)
        wmin_t = -rows[:, :, C_WMIN_T].max(axis=1)  # device stores max(-s)
        wmax_p = rows[:, :, C_WMAX_P].max(axis=1)
        wmin_p = -rows[:, :, C_WMIN_P].max(axis=1)
        sp_sum += -rows[:, :, C_SP].sum()  # device stores ln(sigmoid(-p)) sums
        pt_sum += rows[:, :, C_PT].sum()
        sm_sum += rows[:, :, C_SM].sum()
        # row-boundary smoothness terms
        sig0 = rows[:, :, C_SIG0]
        sigl = rows[:, :, C_SIGL]
        sm_sum += np.abs(sig0[:, 1:] - sigl[:, :-1]).sum()
        t_has = b16[:, 0] > -1e29
        p_has = b16[:, 1] > -1e29
        valid = t_has & p_has
        true_amp = (wmax_t - wmin_t).astype(np.float32)
        pred_amp = (wmax_p - wmin_p).astype(np.float32)
        d = np.abs(true_amp - pred_amp)
        per = np.where(true_amp > 1e-6, d / (true_amp + 1e-6), d)
        amp_sum += np.where(valid, per, 0.0).sum()
    bce = sp_sum / (B * L) - pt_sum / (B * L)
    amp = amp_sum / B
    smooth = sm_sum / (B * (L - 1))
    return np.float32(1.0 * bce + 0.5 * amp + 0.3 * smooth)


def make_in_maps(signals, predictions, targets):
    consts = _make_consts()
    s_all = np.ascontiguousarray(signals[:, 0, :], dtype=np.float32)
    p_all = np.ascontiguousarray(predictions[:, :, 0], dtype=np.float32)
    t_all = np.ascontiguousarray(targets[:, :, 0], dtype=np.float32)
    in_maps = []
    for i in range(N_CORES):
        sl = slice(i * BPC, (i + 1) * BPC)
        m = {
            "s": s_all[sl].reshape(P, F),
            "p": p_all[sl].reshape(P, F),
            "t": t_all[sl].reshape(P, F),
        }
        m.update(consts)
        in_maps.append(m)
    return in_maps


def kernel(signals, predictions, targets):
    nc = _get_nc()
    in_maps = make_in_maps(signals, predictions, targets)
    res = run_bass_kernel_spmd(nc, in_maps, core_ids=list(range(N_CORES)))
    return host_combine(res.results)


# revision 20
# speedup vs baseline: 1.2942x; 1.2942x over previous
"""AmpPerLoss distributed Trainium2 kernel.

Data-parallel over the batch axis: 128 samples split across 8 NeuronCores
(16 samples each). Per core, each sample's length-100000 row is laid out as
8 SBUF partitions x 12500 elements, so a core's shard is [128, 12500].

Per-core device work:
  - BCE partials: sum(softplus(p)) via ACT accumulate, sum(p*t) via a fused
    DVE multiply+add-reduce.
  - Window bounds: m = (t > 0.5) / (p > 0); first/last set index via
    subtiled fused multiply+max-reduce against small index ramps; tiny
    fixup algebra; cross-partition (per-sample) combine via DRAM-bounce DMA.
  - Windowed amplitude: 4x tensor_mask_reduce on s and -s with per-row
    [start, end) bounds.
  - Smoothness partials: shifted subtract on sigmoid(p) + ACT Abs
    accumulate (row-boundary pairs finished on host).

The host reduces the 8 cores' partial stats to the final scalar loss
(the "all-reduce" is three scalars).
"""

import sys

if "/opt/trn_rl_repo" not in sys.path:
    sys.path.insert(0, "/opt/trn_rl_repo")

import numpy as np

import concourse.bass as bass
import concourse.bacc as bacc
import concourse.tile as tile
import concourse.mybir as mybir
from concourse.bass_utils import run_bass_kernel_spmd
from concourse import dve_ops

N_CORES = 8
B, L = 128, 100000
BPC = B // N_CORES          # samples per core
CHUNKS = 8                  # partitions per sample
P = BPC * CHUNKS            # 128 partitions
F = L // CHUNKS             # 12500 free elements per row
NB = 50                     # bound-search subtiles per row
SUB = F // NB               # 250 (fits exactly in bf16 integers)
BIGF = 1.0e30
FMIN = -3.0e38

F32 = mybir.dt.float32
BF16 = mybir.dt.bfloat16
Alu = mybir.AluOpType
Act = mybir.ActivationFunctionType
AX = mybir.AxisListType

# stats column layout
C_WMAX_T, C_WMIN_T, C_WMAX_P, C_WMIN_P = 0, 1, 2, 3
C_SP, C_PT, C_SM = 4, 5, 6
C_SIG0, C_SIGL = 7, 8
NSTAT = 16


def build_nc(n_cores=N_CORES, f=F, nb=NB):
    sub = f // nb
    nc = bacc.Bacc("TRN2", target_bir_lowering=False, debug=False,
                   num_devices=n_cores)

    t_ext = nc.dram_tensor("t", [P, f], BF16, kind="ExternalInput")
    p_ext = nc.dram_tensor("p", [P, f], BF16, kind="ExternalInput")
    s_ext = nc.dram_tensor("s", [P, f], F32, kind="ExternalInput")
    asc_ext = nc.dram_tensor("asc", [P, sub], BF16, kind="ExternalInput")
    desc_ext = nc.dram_tensor("desc", [P, sub], BF16, kind="ExternalInput")
    korr_ext = nc.dram_tensor("korr", [P, 4 * nb], F32, kind="ExternalInput")
    sgn_ext = nc.dram_tensor("sgn", [P, 4 * nb], F32, kind="ExternalInput")
    bigs_ext = nc.dram_tensor("bigs", [P, 4 * nb], F32, kind="ExternalInput")
    off_ext = nc.dram_tensor("off", [P, 1], F32, kind="ExternalInput")

    stats_ext = nc.dram_tensor("stats", [P, NSTAT], F32, kind="ExternalOutput")
    b16_ext = nc.dram_tensor("b16", [BPC, 4], F32, kind="ExternalOutput")

    # DRAM bounce buffers for the partition-crossing rearranges
    rowvals_b = nc.dram_tensor("rowvals_b", [P, 4], F32)
    bounds_b = nc.dram_tensor("bounds_b", [BPC, 4], F32)

    with tile.TileContext(nc) as tc:
        with tc.tile_pool(name="big", bufs=1) as big, \
             tc.tile_pool(name="small", bufs=1) as small:
            t_sb = big.tile([P, f], BF16, tag="T")
            p_sb = big.tile([P, f], BF16, tag="PZ")
            sig_sb = big.tile([P, f], BF16, tag="C")
            m_sb = big.tile([P, f], BF16, tag="D")

            asc_sb = small.tile([P, sub], BF16, tag="asc")
            desc_sb = small.tile([P, sub], BF16, tag="desc")
            korr_sb = small.tile([P, 4 * nb], F32, tag="korr")
            sgn_sb = small.tile([P, 4 * nb], F32, tag="sgn")
            bigs_sb = small.tile([P, 4 * nb], F32, tag="bigs")
            off_sb = small.tile([P, 1], F32, tag="off")
            enc = small.tile([P, 4 * nb], F32, tag="enc")
            cm = small.tile([P, 4 * nb], F32, tag="cm")
            dm = small.tile([P, 4 * nb], F32, tag="dm")
            a1 = small.tile([P, 4 * nb], F32, tag="a1")
            t1 = small.tile([P, 4 * nb], F32, tag="t1")
            t2 = small.tile([P, 4 * nb], F32, tag="t2")
            cand = small.tile([P, 4 * nb], F32, tag="cand")
            rowvals = small.tile([P, 4], F32, tag="rowvals")
            comb = small.tile([BPC, CHUNKS, 4], F32, tag="comb")
            b16 = small.tile([BPC, 4], F32, tag="b16")
            bc = small.tile([P, 4], F32, tag="bc")
            rowb = small.tile([P, 4], F32, tag="rowb")
            stats = small.tile([P, NSTAT], F32, tag="stats")
            nc.vector.memset(stats[:, :], 0.0)

            # ---- input DMAs (free-dim chunks so each spans 128 partitions)
            nchunk = 10
            assert f % nchunk == 0
            fc = f // nchunk
            for i in range(nchunk):
                sl = slice(i * fc, (i + 1) * fc)
                nc.sync.dma_start(out=t_sb[:, sl], in_=t_ext.ap()[:, sl])
            for i in range(nchunk):
                sl = slice(i * fc, (i + 1) * fc)
                nc.sync.dma_start(out=p_sb[:, sl], in_=p_ext.ap()[:, sl])
            nc.sync.dma_start(out=asc_sb, in_=asc_ext.ap())
            nc.sync.dma_start(out=desc_sb, in_=desc_ext.ap())
            nc.sync.dma_start(out=korr_sb, in_=korr_ext.ap())
            nc.sync.dma_start(out=sgn_sb, in_=sgn_ext.ap())
            nc.sync.dma_start(out=bigs_sb, in_=bigs_ext.ap())
            nc.sync.dma_start(out=off_sb, in_=off_ext.ap())

            # ---- BCE: sum p*t via the custom-DVE fused multiply+sum-reduce;
            # softplus via softplus(p) = -ln(sigmoid(-p)) on ACT.
            nc.vector._custom_dve(
                dve_ops.TENSOR_TENSOR_REDUCE,
                out=m_sb[:, :], in0=p_sb[:, :], in1=t_sb[:, :],
                s0=0.0, s1=1.0,
                accum_out=stats[:, C_PT:C_PT + 1])

            # ---- sigmoid for smoothness
            nc.scalar.activation(out=sig_sb[:, :], in_=p_sb[:, :],
                                 func=Act.Sigmoid)

            # ---- bound encodes: for mask in {t>0.5, p>0}, per subtile k:
            #   enc_hi[k] = max(m * (j+1)),  enc_lo[k] = max(m * (SUB-j))
            # One bf16 3-dim multiply (ramp broadcast across subtiles) plus
            # one subtile-view reduce per bound type.
            z_sb = big.tile([P, f], BF16, tag="Z")

            def bound_encodes(src, thr, hi_col0, lo_col0):
                nc.vector.tensor_scalar(out=m_sb[:, :], in0=src[:, :],
                                        scalar1=thr, scalar2=None,
                                        op0=Alu.is_gt)
                mv = m_sb[:, :].rearrange("q (k s) -> q k s", s=sub)
                zv = z_sb[:, :].rearrange("q (k s) -> q k s", s=sub)
                for ramp, col0 in ((asc_sb, hi_col0), (desc_sb, lo_col0)):
                    rv = ramp[:, :].unsqueeze(1).broadcast_to([P, nb, sub])
                    nc.vector.tensor_mul(zv, mv, rv)
                    nc.vector.tensor_reduce(
                        out=enc[:, col0:col0 + nb],
                        in_=zv, axis=AX.X, op=Alu.max)

            bound_encodes(t_sb, 0.5, 0, 2 * nb)
            bound_encodes(p_sb, 0.0, nb, 3 * nb)

            # softplus(p) = -ln(sigmoid(-p)); u reuses t's (now dead) slot.
            # stats[C_SP] holds the NEGATED softplus sum; host negates back.
            u_sb = big.tile([P, f], F32, tag="T")
            nc.scalar.activation(out=u_sb[:, :], in_=p_sb[:, :],
                                 func=Act.Sigmoid, scale=-1.0)
            nc.scalar.activation(out=m_sb[:, :], in_=u_sb[:, :],
                                 func=Act.Ln,
                                 accum_out=stats[:, C_SP:C_SP + 1])

            # ---- fixups on enc [P, 4*nb]:
            # hi cols: cand = j_last_in_row = enc-1 + k*SUB   (korr = k*SUB-1)
            # lo cols: cand = j_first_in_row = k*SUB+SUB - enc (korr = k*SUB+SUB)
            # empty subtile (enc==0) -> -+BIGF
            nc.vector.tensor_scalar(out=cm[:, :], in0=enc[:, :], scalar1=0.0,
                                    scalar2=None, op0=Alu.is_gt)
            nc.vector.tensor_scalar(out=dm[:, :], in0=enc[:, :], scalar1=0.0,
                                    scalar2=None, op0=Alu.is_le)
            nc.vector.tensor_mul(a1[:, :], enc[:, :], sgn_sb[:, :])
            nc.vector.tensor_add(a1[:, :], a1[:, :], korr_sb[:, :])
            nc.vector.tensor_mul(t1[:, :], cm[:, :], a1[:, :])
            nc.vector.tensor_mul(t2[:, :], dm[:, :], bigs_sb[:, :])
            nc.vector.tensor_add(cand[:, :], t1[:, :], t2[:, :])

            # ---- per-row reduce over subtiles -> rowvals [P,4]
            # cols: 0 hi_t (max), 1 hi_p (max), 2 lo_t (min), 3 lo_p (min)
            for i, op in ((0, Alu.max), (1, Alu.max), (2, Alu.min), (3, Alu.min)):
                nc.vector.tensor_reduce(
                    out=rowvals[:, i:i + 1],
                    in_=cand[:, i * nb:(i + 1) * nb],
                    axis=AX.X, op=op)
            # row-local -> global position
            nc.vector.tensor_scalar(out=rowvals[:, :], in0=rowvals[:, :],
                                    scalar1=off_sb[:, 0:1], scalar2=None,
                                    op0=Alu.add)

            # ---- per-sample combine via DRAM bounce
            nc.sync.dma_start(out=rowvals_b.ap(), in_=rowvals[:, :])
            nc.sync.dma_start(
                out=comb[:, :, :],
                in_=rowvals_b.ap().rearrange("(b c) k -> b c k", c=CHUNKS))
            combv = comb[:, :, :].rearrange("b c k -> b k c")
            nc.vector.tensor_reduce(out=b16[:, 0:2], in_=combv[:, 0:2, :],
                                    axis=AX.X, op=Alu.max)
            nc.vector.tensor_reduce(out=b16[:, 2:4], in_=combv[:, 2:4, :],
                                    axis=AX.X, op=Alu.min)
            # end = hi + 1 (half-open)
            nc.vector.tensor_scalar(out=b16[:, 0:2], in0=b16[:, 0:2],
                                    scalar1=1.0, scalar2=None, op0=Alu.add)
            nc.sync.dma_start(out=b16_ext.ap(), in_=b16[:, :])

            # ---- broadcast bounds back to rows via DRAM bounce
            nc.sync.dma_start(out=bounds_b.ap(), in_=b16[:, :])
            bap = bounds_b.ap()
            src_bc = bass.AP(tensor=bap.tensor, offset=bap.offset,
                             ap=[[4, BPC], [0, CHUNKS], [1, 4]])
            nc.sync.dma_start(out=bc[:, :], in_=src_bc)
            # row-local bounds: subtract row offset
            nc.vector.tensor_scalar(out=rowb[:, :], in0=bc[:, :],
                                    scalar1=off_sb[:, 0:1], scalar2=None,
                                    op0=Alu.subtract)
            # rowb cols: 0 end_t, 1 end_p, 2 start_t, 3 start_p

            # ---- smoothness: diff of sigmoid within rows, ACT Abs accum
            nc.vector.tensor_sub(m_sb[:, 0:f - 1], sig_sb[:, 1:f],
                                 sig_sb[:, 0:f - 1])
            nc.scalar.activation(out=m_sb[:, 0:f - 1], in_=m_sb[:, 0:f - 1],
                                 func=Act.Abs,
                                 accum_out=stats[:, C_SM:C_SM + 1])
            # boundary sigmoid values for host
            nc.vector.tensor_copy(stats[:, C_SIG0:C_SIG0 + 1], sig_sb[:, 0:1])
            nc.vector.tensor_copy(stats[:, C_SIGL:C_SIGL + 1],
                                  sig_sb[:, f - 1:f])

            # ---- phase 2: windowed amplitude over s
            s_sb = big.tile([P, f], F32, tag="T")
            sneg_sb = big.tile([P, f], F32, tag="PZ")
            for i in range(nchunk):
                sl = slice(i * fc, (i + 1) * fc)
                nc.sync.dma_start(out=s_sb[:, sl], in_=s_ext.ap()[:, sl])
            nc.vector.tensor_scalar(out=sneg_sb[:, :], in0=s_sb[:, :],
                                    scalar1=-1.0, scalar2=None, op0=Alu.mult)

            # custom TENSOR_MASK_REDUCE: accum = max(s1, max(select(mask,
            # in0, -FLT_MAX) * imm2)); mask is [s0, in1) per row. The min
            # columns hold max(-s) per row; the host negates them back.
            nc.vector._custom_dve(
                dve_ops.TENSOR_MASK_REDUCE,
                out=sig_sb[:, :], in0=s_sb[:, :], in1=rowb[:, 0:1],
                s0=rowb[:, 2:3], s1=FMIN, imm2=1.0,
                accum_out=stats[:, C_WMAX_T:C_WMAX_T + 1])
            nc.vector._custom_dve(
                dve_ops.TENSOR_MASK_REDUCE,
                out=m_sb[:, :], in0=sneg_sb[:, :], in1=rowb[:, 0:1],
                s0=rowb[:, 2:3], s1=FMIN, imm2=1.0,
                accum_out=stats[:, C_WMIN_T:C_WMIN_T + 1])
            nc.vector._custom_dve(
                dve_ops.TENSOR_MASK_REDUCE,
                out=sig_sb[:, :], in0=s_sb[:, :], in1=rowb[:, 1:2],
                s0=rowb[:, 3:4], s1=FMIN, imm2=1.0,
                accum_out=stats[:, C_WMAX_P:C_WMAX_P + 1])
            nc.vector._custom_dve(
                dve_ops.TENSOR_MASK_REDUCE,
                out=m_sb[:, :], in0=sneg_sb[:, :], in1=rowb[:, 1:2],
                s0=rowb[:, 3:4], s1=FMIN, imm2=1.0,
                accum_out=stats[:, C_WMIN_P:C_WMIN_P + 1])

            nc.sync.dma_start(out=stats_ext.ap(), in_=stats[:, :])

    nc.compile()
    return nc


_NC_CACHE = {}


def _get_nc():
    if "nc" not in _NC_CACHE:
        _NC_CACHE["nc"] = build_nc()
    return _NC_CACHE["nc"]


def _make_consts(f=F, nb=NB):
    import ml_dtypes
    sub = f // nb
    bf = ml_dtypes.bfloat16
    asc = np.broadcast_to(np.arange(1, sub + 1, dtype=bf), (P, sub))
    desc = np.broadcast_to(np.arange(sub, 0, -1, dtype=bf), (P, sub))
    k = np.arange(nb, dtype=np.float32)
    korr_hi = k * sub - 1.0
    korr_lo = k * sub + sub
    korr = np.concatenate([korr_hi, korr_hi, korr_lo, korr_lo])
    korr = np.broadcast_to(korr.astype(np.float32), (P, 4 * nb))
    sgn = np.concatenate([np.ones(2 * nb), -np.ones(2 * nb)])
    sgn = np.broadcast_to(sgn.astype(np.float32), (P, 4 * nb))
    bigs = np.concatenate([np.full(2 * nb, -BIGF), np.full(2 * nb, BIGF)])
    bigs = np.broadcast_to(bigs.astype(np.float32), (P, 4 * nb))
    off = ((np.arange(P) % CHUNKS) * f).astype(np.float32).reshape(P, 1)
    return {
        "asc": np.ascontiguousarray(asc),
        "desc": np.ascontiguousarray(desc),
        "korr": np.ascontiguousarray(korr),
        "sgn": np.ascontiguousarray(sgn),
        "bigs": np.ascontiguousarray(bigs),
        "off": off,
    }


def host_combine(results):
    """results: list of dicts with 'stats' [P,16] and 'b16' [BPC,4]."""
    sp_sum = 0.0
    pt_sum = 0.0
    sm_sum = 0.0
    amp_sum = 0.0
    for res in results:
        stats = res["stats"].astype(np.float64)
        b16 = res["b16"].astype(np.float64)
        rows = stats.reshape(BPC, CHUNKS, NSTAT)
        wmax_t = rows[:, :, C_WMAX_T].max(axis=1)
        wmin_t = -rows[:, :, C_WMIN_T].max(axis=1)  # device stores max(-s)
        wmax_p = rows[:, :, C_WMAX_P].max(axis=1)
        wmin_p = -rows[:, :, C_WMIN_P].max(axis=1)
        sp_sum += -rows[:, :, C_SP].sum()  # device stores ln(sigmoid(-p)) sums
        pt_sum += rows[:, :, C_PT].sum()
        sm_sum += rows[:, :, C_SM].sum()
        # row-boundary smoothness terms
        sig0 = rows[:, :, C_SIG0]
        sigl = rows[:, :, C_SIGL]
        sm_sum += np.abs(sig0[:, 1:] - sigl[:, :-1]).sum()
        t_has = b16[:, 0] > -1e29
        p_has = b16[:, 1] > -1e29
        valid = t_has & p_has
        true_amp = (wmax_t - wmin_t).astype(np.float32)
        pred_amp = (wmax_p - wmin_p).astype(np.float32)
        d = np.abs(true_amp - pred_amp)
        per = np.where(true_amp > 1e-6, d / (true_amp + 1e-6), d)
        amp_sum += np.where(valid, per, 0.0).sum()
    bce = sp_sum / (B * L) - pt_sum / (B * L)
    amp = amp_sum / B
    smooth = sm_sum / (B * (L - 1))
    return np.float32(1.0 * bce + 0.5 * amp + 0.3 * smooth)


def make_in_maps(signals, predictions, targets):
    import ml_dtypes
    bf = ml_dtypes.bfloat16
    consts = _make_consts()
    s_all = np.ascontiguousarray(signals[:, 0, :], dtype=np.float32)
    p_all = np.ascontiguousarray(predictions[:, :, 0]).astype(bf)
    t_all = np.ascontiguousarray(targets[:, :, 0]).astype(bf)
    in_maps = []
    for i in range(N_CORES):
        sl = slice(i * BPC, (i + 1) * BPC)
        m = {
            "s": s_all[sl].reshape(P, F),
            "p": p_all[sl].reshape(P, F),
            "t": t_all[sl].reshape(P, F),
        }
        m.update(consts)
        in_maps.append(m)
    return in_maps


def kernel(signals, predictions, targets):
    nc = _get_nc()
    in_maps = make_in_maps(signals, predictions, targets)
    res = run_bass_kernel_spmd(nc, in_maps, core_ids=list(range(N_CORES)))
    return host_combine(res.results)


# revision 26
# speedup vs baseline: 1.7964x; 1.3881x over previous
"""AmpPerLoss distributed Trainium2 kernel (v2: block stats + gather refine).

Data-parallel over the batch axis: 128 samples across 8 NeuronCores
(16 each). Per core each sample's 100000-length row spans 8 SBUF
partitions x 12500, so a shard is [128, 12500]. DRAM inputs are stored
as [128, 10, 1280]: 10 blocks of 1250 padded to 1280 so any block is a
256B-aligned dma_gather row.

Device work per core:
  - BCE: sum softplus(p) = -sum ln(sigmoid(-p)) on ACT; sum p*t on the
    TensorEngine (chunked accumulating matmuls; diagonal extracted via an
    identity mask and row-reduce).
  - Window bounds at block granularity: per-block maxes of t/p (DVE
    subtile reduces), tiny encode/combine algebra, one DRAM-bounce
    round trip for the per-sample combine + broadcast.
  - Exact refinement: dma_gather the 4 boundary blocks per sample and
    find exact bounds/edge extremes inside them (all partition-local).
  - Windowed amplitude: interior from per-block min/max of s via tiny
    masked reduces; edges from the gathered s blocks.
  - Smoothness: shifted subtract of sigmoid(p) + ACT Abs accumulate;
    row-boundary pairs finished on host.

The host reduces the 8 cores' partial stats to the final scalar.
"""

import sys

if "/opt/trn_rl_repo" not in sys.path:
    sys.path.insert(0, "/opt/trn_rl_repo")

from contextlib import ExitStack

import numpy as np

import concourse.bass as bass
import concourse.bacc as bacc
import concourse.tile as tile
import concourse.mybir as mybir
from concourse.bass_utils import run_bass_kernel_spmd
from concourse import dve_ops

N_CORES = 8
B, L = 128, 100000
BPC = B // N_CORES          # samples per core
CHUNKS = 8                  # partitions per sample
P = BPC * CHUNKS            # 128 partitions
F = L // CHUNKS             # 12500 free elements per row
NB = 10                     # blocks per row
SUB = F // NB               # 1250
PB = 1280                   # padded block length in DRAM (256B aligned rows)
BIGF = 1.0e30
FMIN = -3.0e38
MMW = 128                   # matmul chunk width for the p*t diagonal trick

F32 = mybir.dt.float32
BF16 = mybir.dt.bfloat16
I16 = mybir.dt.int16
Alu = mybir.AluOpType
Act = mybir.ActivationFunctionType
AX = mybir.AxisListType

# stats column layout ([P, NSTAT] per-row output)
C_WMAX_T, C_WMIN_T, C_WMAX_P, C_WMIN_P = 0, 1, 2, 3   # interior extremes
C_SP, C_PT, C_SM = 4, 5, 6
C_SIG0, C_SIGL = 7, 8
NSTAT = 16


def build_nc(n_cores=N_CORES, use_mm=True, use_gather=True):
    nc = bacc.Bacc("TRN2", target_bir_lowering=False, debug=False,
                   num_devices=n_cores)

    t_ext = nc.dram_tensor("t", [P, NB, PB], BF16, kind="ExternalInput")
    p_ext = nc.dram_tensor("p", [P, NB, PB], BF16, kind="ExternalInput")
    s_ext = nc.dram_tensor("s", [P, NB, PB], F32, kind="ExternalInput")
    ascB_ext = nc.dram_tensor("ascB", [P, NB], F32, kind="ExternalInput")
    descB_ext = nc.dram_tensor("descB", [P, NB], F32, kind="ExternalInput")
    korrB_ext = nc.dram_tensor("korrB", [P, 4], F32, kind="ExternalInput")
    sgnB_ext = nc.dram_tensor("sgnB", [P, 4], F32, kind="ExternalInput")
    bigsB_ext = nc.dram_tensor("bigsB", [P, 4], F32, kind="ExternalInput")
    offB_ext = nc.dram_tensor("offB", [P, 1], F32, kind="ExternalInput")
    b80_ext = nc.dram_tensor("b80", [P, 1], F32, kind="ExternalInput")
    ramp_ext = nc.dram_tensor("ramp", [P, PB], F32, kind="ExternalInput")
    ident_ext = nc.dram_tensor("ident", [P, MMW], BF16, kind="ExternalInput")

    stats_ext = nc.dram_tensor("stats", [P, NSTAT], F32, kind="ExternalOutput")
    b16_ext = nc.dram_tensor("b16", [BPC, 4], F32, kind="ExternalOutput")
    edge_ext = nc.dram_tensor("edge", [64, 2], F32, kind="ExternalOutput")

    # DRAM bounce buffers for partition-crossing rearranges
    rowvals_b = nc.dram_tensor("rowvals_b", [P, 4], F32)
    bounds_b = nc.dram_tensor("bounds_b", [BPC, 4], F32)
    idx_b = nc.dram_tensor("idx_b", [BPC, 4], F32)
    encs_b = nc.dram_tensor("encs_b", [P, 2], F32)
    sbnd_b = nc.dram_tensor("sbnd_b", [BPC, 8], F32)

    ctx = ExitStack()
    with tile.TileContext(nc) as tc, ctx:
        big = ctx.enter_context(tc.tile_pool(name="big", bufs=1))
        small = ctx.enter_context(tc.tile_pool(name="small", bufs=1))
        psum_pool = ctx.enter_context(
            tc.tile_pool(name="psum", bufs=1, space="PSUM"))

        t_sb = big.tile([P, F], BF16, tag="T")
        p_sb = big.tile([P, F], BF16, tag="PZ")
        sig_sb = big.tile([P, F], BF16, tag="C")
        dump_sb = big.tile([P, F], BF16, tag="D")

        ascB = small.tile([P, NB], F32, tag="ascB")
        descB = small.tile([P, NB], F32, tag="descB")
        korrB = small.tile([P, 4], F32, tag="korrB")
        sgnB = small.tile([P, 4], F32, tag="sgnB")
        bigsB = small.tile([P, 4], F32, tag="bigsB")
        offB = small.tile([P, 1], F32, tag="offB")
        b80 = small.tile([P, 1], F32, tag="b80")
        ramp = small.tile([P, PB], F32, tag="ramp")
        ident = small.tile([P, MMW], BF16, tag="ident")
        for sb, ext in ((ascB, ascB_ext), (descB, descB_ext),
                        (korrB, korrB_ext), (sgnB, sgnB_ext),
                        (bigsB, bigsB_ext), (offB, offB_ext),
                        (b80, b80_ext), (ramp, ramp_ext), (ident, ident_ext)):
            nc.sync.dma_start(out=sb, in_=ext.ap())

        stats = small.tile([P, NSTAT], F32, tag="stats")
        nc.vector.memset(stats[:, :], 0.0)

        bmax_t = small.tile([P, NB], F32, tag="bmax_t")
        bmax_p = small.tile([P, NB], F32, tag="bmax_p")
        bmax_s = small.tile([P, NB], F32, tag="bmax_s")
        bmin_s = small.tile([P, NB], F32, tag="bmin_s")

        # ---- load t/p and per-block maxes (chunked for DMA overlap)
        for k in range(NB):
            nc.sync.dma_start(out=t_sb[:, k * SUB:(k + 1) * SUB],
                              in_=t_ext.ap()[:, k, 0:SUB])
        for k in range(NB):
            nc.sync.dma_start(out=p_sb[:, k * SUB:(k + 1) * SUB],
                              in_=p_ext.ap()[:, k, 0:SUB])
        tv = t_sb[:, :].rearrange("q (k s) -> q k s", s=SUB)
        pv = p_sb[:, :].rearrange("q (k s) -> q k s", s=SUB)
        nc.vector.tensor_reduce(out=bmax_t[:, :], in_=tv, axis=AX.X, op=Alu.max)
        nc.vector.tensor_reduce(out=bmax_p[:, :], in_=pv, axis=AX.X, op=Alu.max)

        # ---- p*t on TensorE: accumulate p_chunk^T @ t_chunk into PSUM,
        # then pull the diagonal with an identity mask.
        use_mm_ = use_mm
        psum = psum_pool.tile([MMW, MMW], F32)
        nmm = (F + MMW - 1) // MMW if use_mm_ else 0
        # first and last chunks must be full-width so the PSUM accumulation
        # group opens/closes over the whole region; the partial chunk (if
        # any) goes in between.
        order = ([0] + list(range(nmm - 1, 0, -1))) if nmm else []
        for i, c in enumerate(order):
            w = min(MMW, F - c * MMW)
            nc.tensor.matmul(out=psum[0:w, 0:w],
                             lhsT=p_sb[:, c * MMW:c * MMW + w],
                             rhs=t_sb[:, c * MMW:c * MMW + w],
                             start=(i == 0), stop=(i == nmm - 1))
        if use_mm_:
            diag = small.tile([P, MMW], F32, tag="diag")
            nc.vector.tensor_mul(diag[:, :], psum[:, :], ident[:, :])
            nc.vector.tensor_reduce(out=stats[:, C_PT:C_PT + 1], in_=diag[:, :],
                                    axis=AX.X, op=Alu.add)
        else:
            nc.vector._custom_dve(
                dve_ops.TENSOR_TENSOR_REDUCE,
                out=dump_sb[:, :], in0=p_sb[:, :], in1=t_sb[:, :],
                s0=0.0, s1=1.0, accum_out=stats[:, C_PT:C_PT + 1])

        # ---- ACT: sigmoid, softplus pieces, smoothness abs-accum
        nc.scalar.activation(out=sig_sb[:, :], in_=p_sb[:, :], func=Act.Sigmoid)
        u_sb = big.tile([P, F], F32, tag="T")         # reuses t's slot
        nc.scalar.activation(out=u_sb[:, :], in_=p_sb[:, :],
                             func=Act.Sigmoid, scale=-1.0)
        nc.scalar.activation(out=dump_sb[:, :], in_=u_sb[:, :], func=Act.Ln,
                             accum_out=stats[:, C_SP:C_SP + 1])
        nc.vector.tensor_sub(dump_sb[:, 0:F - 1], sig_sb[:, 1:F],
                             sig_sb[:, 0:F - 1])
        nc.scalar.activation(out=dump_sb[:, 0:F - 1], in_=dump_sb[:, 0:F - 1],
                             func=Act.Abs, accum_out=stats[:, C_SM:C_SM + 1])
        nc.vector.tensor_copy(stats[:, C_SIG0:C_SIG0 + 1], sig_sb[:, 0:1])
        nc.vector.tensor_copy(stats[:, C_SIGL:C_SIGL + 1], sig_sb[:, F - 1:F])

        # ---- s load + per-block min/max
        s_sb = big.tile([P, F], F32, tag="PZ")        # reuses p's slot
        for k in range(NB):
            nc.sync.dma_start(out=s_sb[:, k * SUB:(k + 1) * SUB],
                              in_=s_ext.ap()[:, k, 0:SUB])
        sv = s_sb[:, :].rearrange("q (k s) -> q k s", s=SUB)
        nc.vector.tensor_reduce(out=bmax_s[:, :], in_=sv, axis=AX.X, op=Alu.max)
        nc.vector.tensor_reduce(out=bmin_s[:, :], in_=sv, axis=AX.X, op=Alu.min)

        # ---- block-level bound search (all tiny)
        anyt = small.tile([P, NB], F32, tag="anyt")
        anyp = small.tile([P, NB], F32, tag="anyp")
        nc.vector.tensor_scalar(out=anyt[:, :], in0=bmax_t[:, :],
                                scalar1=0.5, scalar2=None, op0=Alu.is_gt)
        nc.vector.tensor_scalar(out=anyp[:, :], in0=bmax_p[:, :],
                                scalar1=0.0, scalar2=None, op0=Alu.is_gt)
        encB = small.tile([P, 4], F32, tag="encB")     # [hi_t, hi_p, lo_t, lo_p]
        ze = small.tile([P, NB], F32, tag="ze")
        for i, (src, rmp) in enumerate(((anyt, ascB), (anyp, ascB),
                                        (anyt, descB), (anyp, descB))):
            nc.vector.tensor_mul(ze[:, :], src[:, :], rmp[:, :])
            nc.vector.tensor_reduce(out=encB[:, i:i + 1], in_=ze[:, :],
                                    axis=AX.X, op=Alu.max)
        # fixups: hi cols: cand = enc-1 + 10c; lo: cand = 10-enc + 10c
        cm = small.tile([P, 4], F32, tag="cm")
        dm = small.tile([P, 4], F32, tag="dm")
        a1 = small.tile([P, 4], F32, tag="a1")
        t1 = small.tile([P, 4], F32, tag="t1")
        t2 = small.tile([P, 4], F32, tag="t2")
        rowvals = small.tile([P, 4], F32, tag="rowvals")
        nc.vector.tensor_scalar(out=cm[:, :], in0=encB[:, :], scalar1=0.0,
                                scalar2=None, op0=Alu.is_gt)
        nc.vector.tensor_scalar(out=dm[:, :], in0=encB[:, :], scalar1=0.0,
                                scalar2=None, op0=Alu.is_le)
        nc.vector.tensor_mul(a1[:, :], encB[:, :], sgnB[:, :])
        nc.vector.tensor_add(a1[:, :], a1[:, :], korrB[:, :])
        nc.vector.tensor_mul(t1[:, :], cm[:, :], a1[:, :])
        nc.vector.tensor_mul(t2[:, :], dm[:, :], bigsB[:, :])
        nc.vector.tensor_add(rowvals[:, :], t1[:, :], t2[:, :])
        # local block k -> global block id (+10c); +-BIG rows stay huge
        nc.vector.tensor_scalar(out=rowvals[:, :], in0=rowvals[:, :],
                                scalar1=offB[:, 0:1], scalar2=None, op0=Alu.add)

        # ---- per-sample combine via DRAM bounce -> b16blk [16,4]
        comb = small.tile([BPC, CHUNKS, 4], F32, tag="comb")
        b16blk = small.tile([BPC, 4], F32, tag="b16blk")
        nc.sync.dma_start(out=rowvals_b.ap(), in_=rowvals[:, :])
        nc.sync.dma_start(
            out=comb[:, :, :],
            in_=rowvals_b.ap().rearrange("(b c) k -> b c k", c=CHUNKS))
        combv = comb[:, :, :].rearrange("b c k -> b k c")
        nc.vector.tensor_reduce(out=b16blk[:, 0:2], in_=combv[:, 0:2, :],
                                axis=AX.X, op=Alu.max)
        nc.vector.tensor_reduce(out=b16blk[:, 2:4], in_=combv[:, 2:4, :],
                                axis=AX.X, op=Alu.min)
        nc.sync.dma_start(out=b16_ext.ap(), in_=b16blk[:, :])

        # ---- broadcast block bounds to rows: bc [128,4] global ids
        bc = small.tile([P, 4], F32, tag="bc")
        nc.sync.dma_start(out=bounds_b.ap(), in_=b16blk[:, :])
        bap = bounds_b.ap()
        nc.sync.dma_start(out=bc[:, :], in_=bass.AP(
            tensor=bap.tensor, offset=bap.offset, ap=[[4, BPC], [0, CHUNKS], [1, 4]]))

        # ---- interior extremes from block stats (masked block reduces)
        ibs = small.tile([P, 2], F32, tag="ibs")
        ibe = small.tile([P, 2], F32, tag="ibe")
        nc.vector.tensor_scalar(out=ibs[:, :], in0=bc[:, 2:4],
                                scalar1=offB[:, 0:1], scalar2=1.0,
                                op0=Alu.subtract, op1=Alu.add)
        nc.vector.tensor_scalar(out=ibe[:, :], in0=bc[:, 0:2],
                                scalar1=offB[:, 0:1], scalar2=None,
                                op0=Alu.subtract)
        nc.vector.tensor_tensor(out=ibs[:, :], in0=ibs[:, :], in1=ibe[:, :],
                                op=Alu.min)
        negb = small.tile([P, NB], F32, tag="negb")
        nc.vector.tensor_scalar(out=negb[:, :], in0=bmin_s[:, :],
                                scalar1=-1.0, scalar2=None, op0=Alu.mult)
        bdump = small.tile([P, NB], F32, tag="bdump")
        for (data, scol, ccol) in ((bmax_s, 0, C_WMAX_T), (negb, 0, C_WMIN_T),
                                   (bmax_s, 1, C_WMAX_P), (negb, 1, C_WMIN_P)):
            nc.vector._custom_dve(
                dve_ops.TENSOR_MASK_REDUCE,
                out=bdump[:, :], in0=data[:, :], in1=ibe[:, scol:scol + 1],
                s0=ibs[:, scol:scol + 1], s1=FMIN, imm2=1.0,
                accum_out=stats[:, ccol:ccol + 1])

        # ---- gather indices: idx = clamp(80b + g, 0, 1279), replicated
        idx4 = small.tile([BPC, 4], F32, tag="idx4")   # [lo_t, hi_t, lo_p, hi_p]
        for dst_c, src_c in ((0, 2), (1, 0), (2, 3), (3, 1)):
            nc.vector.tensor_copy(idx4[:, dst_c:dst_c + 1],
                                  b16blk[:, src_c:src_c + 1])
        nc.vector.tensor_scalar(out=idx4[:, :], in0=idx4[:, :],
                                scalar1=b80[0:BPC, 0:1], scalar2=0.0,
                                op0=Alu.add, op1=Alu.max)
        nc.vector.tensor_scalar(out=idx4[:, :], in0=idx4[:, :],
                                scalar1=1279.0, scalar2=None, op0=Alu.min)
        idxr = small.tile([P, 4], F32, tag="idxr")
        nc.sync.dma_start(out=idx_b.ap(), in_=idx4[:, :])
        iap = idx_b.ap()
        nc.sync.dma_start(out=idxr[:, :], in_=bass.AP(
            tensor=iap.tensor, offset=iap.offset,
            ap=[[0, CHUNKS], [4, BPC], [1, 4]]))
        idx_i = small.tile([P, 4], I16, tag="idx_i")
        nc.vector.tensor_copy(idx_i[:, :], idxr[:, :])

        # ---- gathers (manual DMA semaphores inside critical sections)
        tg = small.tile([P, 1, PB], BF16, tag="tg")
        pg = small.tile([P, 1, PB], BF16, tag="pg")
        sg = small.tile([P, 1, PB], F32, tag="sg")
        if not use_gather:
            nc.vector.memset(tg[:, :, :], 0.0)
            nc.vector.memset(pg[:, :, :], 0.0)
            nc.vector.memset(sg[:, :, :], 0.0)
        if use_gather:
          with tc.tile_critical():
            with nc.semaphore("gsem_t") as gsem:
                nc.gpsimd.dma_gather(
                    out_ap=tg[:, :, :],
                    in_ap=t_ext.ap().rearrange("q k s -> (q k) s"),
                    idxs_ap=idx_i[:, 0:2], num_idxs=2 * BPC,
                    num_idxs_reg=2 * BPC, elem_size=PB).then_inc(gsem, 16)
                nc.gpsimd.wait_ge(gsem, 16)
        if use_gather:
          with tc.tile_critical():
            with nc.semaphore("gsem_p") as gsem:
                nc.gpsimd.dma_gather(
                    out_ap=pg[:, :, :],
                    in_ap=p_ext.ap().rearrange("q k s -> (q k) s"),
                    idxs_ap=idx_i[:, 2:4], num_idxs=2 * BPC,
                    num_idxs_reg=2 * BPC, elem_size=PB).then_inc(gsem, 16)
                nc.gpsimd.wait_ge(gsem, 16)
        if use_gather:
          with tc.tile_critical():
            with nc.semaphore("gsem_s") as gsem:
                nc.gpsimd.dma_gather(
                    out_ap=sg[:, :, :],
                    in_ap=s_ext.ap().rearrange("q k s -> (q k) s"),
                    idxs_ap=idx_i[:, 0:4], num_idxs=4 * BPC,
                    num_idxs_reg=4 * BPC, elem_size=PB).then_inc(gsem, 16)
                nc.gpsimd.wait_ge(gsem, 16)

        # ---- refine exact bounds inside the gathered t/p blocks
        # rows 0:16 lo-blocks (desc ramp -> enc = SUB - pos), 16:32 hi-blocks
        # (asc ramp -> enc = pos + 1)
        refm = small.tile([64, PB], BF16, tag="refm")
        refz = small.tile([64, PB], F32, tag="refz")
        enc2 = small.tile([64, 2], F32, tag="enc2")
        nc.vector.tensor_scalar(out=refm[0:32, :], in0=tg[0:32, 0, :],
                                scalar1=0.5, scalar2=None, op0=Alu.is_gt)
        nc.vector.tensor_mul(refz[0:32, :], refm[0:32, :], ramp[0:32, :])
        nc.vector.tensor_reduce(out=enc2[0:32, 0:1], in_=refz[0:32, :],
                                axis=AX.X, op=Alu.max)
        nc.vector.tensor_scalar(out=refm[0:32, :], in0=pg[0:32, 0, :],
                                scalar1=0.0, scalar2=None, op0=Alu.is_gt)
        nc.vector.tensor_mul(refz[0:32, :], refm[0:32, :], ramp[0:32, :])
        nc.vector.tensor_reduce(out=enc2[0:32, 1:2], in_=refz[0:32, :],
                                axis=AX.X, op=Alu.max)

        # ---- per-sample window bounds for the gathered s blocks
        encs16 = small.tile([BPC, 4], F32, tag="encs16")
        nc.sync.dma_start(out=encs_b.ap()[0:32, :], in_=enc2[0:32, :])
        eap = encs_b.ap()
        nc.sync.dma_start(out=encs16[:, 0:2], in_=bass.AP(
            tensor=eap.tensor, offset=eap.offset, ap=[[2, BPC], [32, 2], [1, 1]]))
        nc.sync.dma_start(out=encs16[:, 2:4], in_=bass.AP(
            tensor=eap.tensor, offset=eap.offset + 1,
            ap=[[2, BPC], [32, 2], [1, 1]]))
        # encs16 cols: [enc_lo_t, enc_hi_t, enc_lo_p, enc_hi_p]
        sb8 = small.tile([BPC, 8], F32, tag="sb8")
        eq2 = small.tile([BPC, 2], F32, tag="eq2")
        tmp2 = small.tile([BPC, 2], F32, tag="tmp2")
        # eq = (g_lo == g_hi) per mask; b16blk cols [hi_t, hi_p, lo_t, lo_p]
        nc.vector.tensor_tensor(out=eq2[:, :], in0=b16blk[:, 2:4],
                                in1=b16blk[:, 0:2], op=Alu.is_equal)
        # starts of lo-rows: SUB - enc_lo  (cols 0=t, 1=p)
        for c, ec in ((0, 0), (1, 2)):
            nc.vector.tensor_scalar(
                out=sb8[:, 4 * c + 0:4 * c + 1], in0=encs16[:, ec:ec + 1],
                scalar1=-1.0, scalar2=float(SUB), op0=Alu.mult, op1=Alu.add)
        # ends of lo-rows: SUB + eq*(enc_hi - SUB)
        for c, ec in ((0, 1), (1, 3)):
            nc.vector.tensor_scalar(
                out=tmp2[:, c:c + 1], in0=encs16[:, ec:ec + 1],
                scalar1=-float(SUB), scalar2=None, op0=Alu.add)
        nc.vector.tensor_mul(tmp2[:, :], tmp2[:, :], eq2[:, :])
        for c in (0, 1):
            nc.vector.tensor_scalar(
                out=sb8[:, 4 * c + 1:4 * c + 2], in0=tmp2[:, c:c + 1],
                scalar1=float(SUB), scalar2=None, op0=Alu.add)
        # starts of hi-rows: eq * start_lo
        for c in (0, 1):
            nc.vector.tensor_mul(sb8[:, 4 * c + 2:4 * c + 3],
                                 eq2[:, c:c + 1], sb8[:, 4 * c + 0:4 * c + 1])
        # ends of hi-rows: enc_hi
        for c, ec in ((0, 1), (1, 3)):
            nc.vector.tensor_copy(sb8[:, 4 * c + 3:4 * c + 4],
                                  encs16[:, ec:ec + 1])
        # sb8 cols: [st_lo_t, en_lo_t, st_hi_t, en_hi_t, st_lo_p, ...]
        # rearrange to per-sg-row [64, 2] (row j = group*16+b; groups:
        # lo_t, hi_t, lo_p, hi_p)
        sbnd = small.tile([64, 2], F32, tag="sbnd")
        nc.sync.dma_start(out=sbnd_b.ap(), in_=sb8[:, :])
        sap = sbnd_b.ap()
        nc.sync.dma_start(out=sbnd[:, :], in_=bass.AP(
            tensor=sap.tensor, offset=sap.offset,
            ap=[[2, 4], [8, BPC], [1, 2]]))

        # ---- edge extremes from gathered s blocks
        sgneg = small.tile([64, PB], F32, tag="sgneg")
        edge = small.tile([64, 2], F32, tag="edge")
        nc.vector.tensor_scalar(out=sgneg[0:64, :], in0=sg[0:64, 0, :],
                                scalar1=-1.0, scalar2=None, op0=Alu.mult)
        edump = small.tile([64, PB], BF16, tag="edump")
        nc.vector._custom_dve(
            dve_ops.TENSOR_MASK_REDUCE,
            out=edump[0:64, :], in0=sg[0:64, 0, :], in1=sbnd[:, 1:2],
            s0=sbnd[:, 0:1], s1=FMIN, imm2=1.0, accum_out=edge[:, 0:1])
        nc.vector._custom_dve(
            dve_ops.TENSOR_MASK_REDUCE,
            out=edump[0:64, :], in0=sgneg[0:64, :], in1=sbnd[:, 1:2],
            s0=sbnd[:, 0:1], s1=FMIN, imm2=1.0, accum_out=edge[:, 1:2])
        nc.sync.dma_start(out=edge_ext.ap(), in_=edge[:, :])

        nc.sync.dma_start(out=stats_ext.ap(), in_=stats[:, :])

    nc.compile()
    return nc


_NC_CACHE = {}


def _get_nc():
    if "nc" not in _NC_CACHE:
        _NC_CACHE["nc"] = build_nc()
    return _NC_CACHE["nc"]


def _make_consts():
    ascB = np.broadcast_to(np.arange(1, NB + 1, dtype=np.float32), (P, NB))
    descB = np.broadcast_to(np.arange(NB, 0, -1, dtype=np.float32), (P, NB))
    korrB = np.broadcast_to(
        np.array([-1.0, -1.0, float(NB), float(NB)], np.float32), (P, 4))
    sgnB = np.broadcast_to(np.array([1.0, 1.0, -1.0, -1.0], np.float32), (P, 4))
    bigsB = np.broadcast_to(
        np.array([-BIGF, -BIGF, BIGF, BIGF], np.float32), (P, 4))
    offB = (float(NB) * (np.arange(P) % CHUNKS)).astype(np.float32).reshape(P, 1)
    b80 = (float(NB * CHUNKS) * np.arange(P)).astype(np.float32).reshape(P, 1)
    ramp = np.zeros((P, PB), np.float32)
    j = np.arange(SUB, dtype=np.float32)
    ramp[0:BPC, 0:SUB] = SUB - j          # desc for lo rows
    ramp[BPC:2 * BPC, 0:SUB] = j + 1      # asc for hi rows
    ident = np.eye(P, MMW, dtype=np.float32)
    import ml_dtypes
    return {
        "ascB": np.ascontiguousarray(ascB),
        "descB": np.ascontiguousarray(descB),
        "korrB": np.ascontiguousarray(korrB),
        "sgnB": np.ascontiguousarray(sgnB),
        "bigsB": np.ascontiguousarray(bigsB),
        "offB": offB,
        "b80": b80,
        "ramp": ramp,
        "ident": ident.astype(ml_dtypes.bfloat16),
    }


def _pad_blocks(arr, dtype):
    """[BPC*CHUNKS, F] -> [P, NB, PB] with zero padding per block."""
    out = np.zeros((P, NB, PB), dtype=dtype)
    out[:, :, 0:SUB] = arr.reshape(P, NB, SUB)
    return out


def host_combine(results):
    sp_sum = 0.0
    pt_sum = 0.0
    sm_sum = 0.0
    amp_sum = 0.0
    for res in results:
        stats = res["stats"].astype(np.float64)
        b16 = res["b16"].astype(np.float64)
        edge = res["edge"].astype(np.float64)
        rows = stats.reshape(BPC, CHUNKS, NSTAT)
        e4 = edge.reshape(4, BPC, 2)   # groups: lo_t, hi_t, lo_p, hi_p
        wmax_t = np.maximum(rows[:, :, C_WMAX_T].max(axis=1),
                            np.maximum(e4[0, :, 0], e4[1, :, 0]))
        wmin_t = np.minimum(-rows[:, :, C_WMIN_T].max(axis=1),
                            np.minimum(-e4[0, :, 1], -e4[1, :, 1]))
        wmax_p = np.maximum(rows[:, :, C_WMAX_P].max(axis=1),
                            np.maximum(e4[2, :, 0], e4[3, :, 0]))
        wmin_p = np.minimum(-rows[:, :, C_WMIN_P].max(axis=1),
                            np.minimum(-e4[2, :, 1], -e4[3, :, 1]))
        sp_sum += -rows[:, :, C_SP].sum()
        pt_sum += rows[:, :, C_PT].sum()
        sm_sum += rows[:, :, C_SM].sum()
        sig0 = rows[:, :, C_SIG0]
        sigl = rows[:, :, C_SIGL]
        sm_sum += np.abs(sig0[:, 1:] - sigl[:, :-1]).sum()
        t_has = b16[:, 0] > -1e29
        p_has = b16[:, 1] > -1e29
        valid = t_has & p_has
        true_amp = (wmax_t - wmin_t).astype(np.float32)
        pred_amp = (wmax_p - wmin_p).astype(np.float32)
        d = np.abs(true_amp - pred_amp)
        per = np.where(true_amp > 1e-6, d / (true_amp + 1e-6), d)
        amp_sum += np.where(valid, per, 0.0).sum()
    bce = sp_sum / (B * L) - pt_sum / (B * L)
    amp = amp_sum / B
    smooth = sm_sum / (B * (L - 1))
    return np.float32(1.0 * bce + 0.5 * amp + 0.3 * smooth)


def make_in_maps(signals, predictions, targets):
    import ml_dtypes
    bf = ml_dtypes.bfloat16
    consts = _make_consts()
    s_all = np.ascontiguousarray(signals[:, 0, :], dtype=np.float32)
    p_all = np.ascontiguousarray(predictions[:, :, 0]).astype(bf)
    t_all = np.ascontiguousarray(targets[:, :, 0]).astype(bf)
    in_maps = []
    for i in range(N_CORES):
        sl = slice(i * BPC, (i + 1) * BPC)
        m = {
            "s": _pad_blocks(s_all[sl].reshape(P, F), np.float32),
            "p": _pad_blocks(p_all[sl].reshape(P, F), bf),
            "t": _pad_blocks(t_all[sl].reshape(P, F), bf),
        }
        m.update(consts)
        in_maps.append(m)
    return in_maps


def kernel(signals, predictions, targets):
    nc = _get_nc()
    in_maps = make_in_maps(signals, predictions, targets)
    res = run_bass_kernel_spmd(nc, in_maps, core_ids=list(range(N_CORES)))
    return host_combine(res.results)


# revision 47
# speedup vs baseline: 2.6707x; 1.4867x over previous
"""AmpPerLoss distributed Trainium2 kernel (v2: block stats + gather refine).

Data-parallel over the batch axis: 128 samples across 8 NeuronCores
(16 each). Per core each sample's 100000-length row spans 8 SBUF
partitions x 12500, so a shard is [128, 12500]. DRAM inputs are stored
as [128, 10, 1280]: 10 blocks of 1250 padded to 1280 so any block is a
256B-aligned dma_gather row.

Device work per core:
  - BCE: sum softplus(p) = -sum ln(sigmoid(-p)) on ACT; sum p*t on the
    TensorEngine (chunked accumulating matmuls; diagonal extracted via an
    identity mask and row-reduce).
  - Window bounds at block granularity: per-block maxes of t/p (DVE
    subtile reduces), tiny encode/combine algebra, one DRAM-bounce
    round trip for the per-sample combine + broadcast.
  - Exact refinement: dma_gather the 4 boundary blocks per sample and
    find exact bounds/edge extremes inside them (all partition-local).
  - Windowed amplitude: interior from per-block min/max of s via tiny
    masked reduces; edges from the gathered s blocks.
  - Smoothness: shifted subtract of sigmoid(p) + ACT Abs accumulate;
    row-boundary pairs finished on host.

The host reduces the 8 cores' partial stats to the final scalar.
"""

import sys

if "/opt/trn_rl_repo" not in sys.path:
    sys.path.insert(0, "/opt/trn_rl_repo")

from contextlib import ExitStack

import numpy as np

import concourse.bass as bass
import concourse.bacc as bacc
import concourse.tile as tile
import concourse.mybir as mybir
from concourse.bass_utils import run_bass_kernel_spmd
from concourse import dve_ops

N_CORES = 8
B, L = 128, 100000
BPC = B // N_CORES          # samples per core
CHUNKS = 8                  # partitions per sample
P = BPC * CHUNKS            # 128 partitions
F = L // CHUNKS             # 12500 free elements per row
NB = 10                     # blocks per row
SUB = F // NB               # 1250
PB = 1280                   # padded block length in DRAM (256B aligned rows)
BIGF = 1.0e30
FMIN = -3.0e38
MMW = 128                   # matmul chunk width for the p*t diagonal trick

F32 = mybir.dt.float32
BF16 = mybir.dt.bfloat16
I16 = mybir.dt.int16
Alu = mybir.AluOpType
Act = mybir.ActivationFunctionType
AX = mybir.AxisListType

# stats column layout ([P, NSTAT] per-row output)
C_WMAX_T, C_WMIN_T, C_WMAX_P, C_WMIN_P = 0, 1, 2, 3   # interior extremes
C_SP, C_PT, C_SM = 4, 5, 6
C_SIG0, C_SIGL = 7, 8
NSTAT = 16


def build_nc(n_cores=N_CORES, use_mm=True, use_gather=True):
    nc = bacc.Bacc("TRN2", target_bir_lowering=False, debug=False,
                   num_devices=n_cores)

    t_ext = nc.dram_tensor("t", [P, F], BF16, kind="ExternalInput")
    p_ext = nc.dram_tensor("p", [P, F], BF16, kind="ExternalInput")
    s_ext = nc.dram_tensor("s", [P, F], BF16, kind="ExternalInput")
    tpad_ext = nc.dram_tensor("tpad", [P * NB, PB], BF16, kind="ExternalInput")
    ppad_ext = nc.dram_tensor("ppad", [P * NB, PB], BF16, kind="ExternalInput")
    spad_ext = nc.dram_tensor("spad", [P * NB, PB], BF16, kind="ExternalInput")
    ascB_ext = nc.dram_tensor("ascB", [P, NB], F32, kind="ExternalInput")
    descB_ext = nc.dram_tensor("descB", [P, NB], F32, kind="ExternalInput")
    korrB_ext = nc.dram_tensor("korrB", [P, 4], F32, kind="ExternalInput")
    sgnB_ext = nc.dram_tensor("sgnB", [P, 4], F32, kind="ExternalInput")
    bigsB_ext = nc.dram_tensor("bigsB", [P, 4], F32, kind="ExternalInput")
    offB_ext = nc.dram_tensor("offB", [P, 1], F32, kind="ExternalInput")
    b80_ext = nc.dram_tensor("b80", [P, 1], F32, kind="ExternalInput")
    ramp_ext = nc.dram_tensor("ramp", [P, PB], F32, kind="ExternalInput")
    ident_ext = nc.dram_tensor("ident", [P, MMW], BF16, kind="ExternalInput")

    stats_ext = nc.dram_tensor("stats", [P, NSTAT], F32, kind="ExternalOutput")
    b16_ext = nc.dram_tensor("b16", [BPC, 4], F32, kind="ExternalOutput")
    edge_ext = nc.dram_tensor("edge", [64, 2], F32, kind="ExternalOutput")

    # DRAM bounce buffers for partition-crossing rearranges
    rowvals_b = nc.dram_tensor("rowvals_b", [P, 4], F32)
    pack_b = nc.dram_tensor("pack_b", [BPC, 8], F32)
    encs_b = nc.dram_tensor("encs_b", [P, 2], F32)
    sbnd_b = nc.dram_tensor("sbnd_b", [BPC, 8], F32)

    ctx = ExitStack()
    with tile.TileContext(nc) as tc, ctx:
        big = ctx.enter_context(tc.tile_pool(name="big", bufs=1))
        small = ctx.enter_context(tc.tile_pool(name="small", bufs=1))
        psum_pool = ctx.enter_context(
            tc.tile_pool(name="psum", bufs=1, space="PSUM"))

        t_sb = big.tile([P, F], BF16, tag="T")
        p_sb = big.tile([P, F], BF16, tag="PZ")
        sig_sb = big.tile([P, F], BF16, tag="C")
        dump_sb = big.tile([P, F], BF16, tag="D")

        ascB = small.tile([P, NB], F32, tag="ascB")
        descB = small.tile([P, NB], F32, tag="descB")
        korrB = small.tile([P, 4], F32, tag="korrB")
        sgnB = small.tile([P, 4], F32, tag="sgnB")
        bigsB = small.tile([P, 4], F32, tag="bigsB")
        offB = small.tile([P, 1], F32, tag="offB")
        b80 = small.tile([P, 1], F32, tag="b80")
        ramp = small.tile([P, PB], F32, tag="ramp")
        ident = small.tile([P, MMW], BF16, tag="ident")
        for sb, ext in ((ascB, ascB_ext), (descB, descB_ext),
                        (korrB, korrB_ext), (sgnB, sgnB_ext),
                        (bigsB, bigsB_ext), (offB, offB_ext),
                        (b80, b80_ext), (ramp, ramp_ext), (ident, ident_ext)):
            nc.sync.dma_start(out=sb, in_=ext.ap())

        stats = small.tile([P, NSTAT], F32, tag="stats")
        nc.vector.memset(stats[:, :], 0.0)

        bmax_t = small.tile([P, NB], F32, tag="bmax_t")
        bmax_p = small.tile([P, NB], F32, tag="bmax_p")
        bmax_s = small.tile([P, NB], F32, tag="bmax_s")
        bmin_s = small.tile([P, NB], F32, tag="bmin_s")

        # ---- load t/p/s in 2-block chunks; per-chunk block maxes so the
        # bound chain starts as soon as the last chunk lands.
        NCH = 5
        fch = F // NCH          # 2500 = 2 blocks
        BPCH = NB // NCH        # blocks per chunk
        s_sb = big.tile([P, F], BF16, tag="S")
        for k in range(NCH):
            sl = slice(k * fch, (k + 1) * fch)
            nc.sync.dma_start(out=p_sb[:, sl], in_=p_ext.ap()[:, sl])
        for k in range(NCH):
            sl = slice(k * fch, (k + 1) * fch)
            nc.sync.dma_start(out=t_sb[:, sl], in_=t_ext.ap()[:, sl])
        for k in range(NCH):
            sl = slice(k * fch, (k + 1) * fch)
            nc.sync.dma_start(out=s_sb[:, sl], in_=s_ext.ap()[:, sl])
        tv = t_sb[:, :].rearrange("q (k s) -> q k s", s=SUB)
        pv = p_sb[:, :].rearrange("q (k s) -> q k s", s=SUB)
        sv = s_sb[:, :].rearrange("q (k s) -> q k s", s=SUB)
        # gpsimd folds each 1250-block in half (pairwise max of the two
        # 625-halves); DVE reduces the halved views - halves the Vector cost
        # of every block-stat reduce.
        HSUB = SUB // 2
        half = big.tile([P, NB, HSUB], BF16, tag="HALF")
        hv = half[:, :, :]
        for k in range(NCH):
            bsl = slice(k * BPCH, (k + 1) * BPCH)
            nc.vector.tensor_max(hv[:, bsl, :], pv[:, bsl, 0:HSUB],
                                 pv[:, bsl, HSUB:SUB])
            nc.vector.tensor_reduce(out=bmax_p[:, bsl], in_=hv[:, bsl, :],
                                    axis=AX.X, op=Alu.max)
        for k in range(NCH):
            bsl = slice(k * BPCH, (k + 1) * BPCH)
            nc.vector.tensor_max(hv[:, bsl, :], tv[:, bsl, 0:HSUB],
                                 tv[:, bsl, HSUB:SUB])
            nc.vector.tensor_reduce(out=bmax_t[:, bsl], in_=hv[:, bsl, :],
                                    axis=AX.X, op=Alu.max)

        # ---- p*t on TensorE: accumulate p_chunk^T @ t_chunk into PSUM,
        # then pull the diagonal with an identity mask.
        use_mm_ = use_mm
        psum = psum_pool.tile([MMW, MMW], F32)
        nmm = (F + MMW - 1) // MMW if use_mm_ else 0
        # first and last chunks must be full-width so the PSUM accumulation
        # group opens/closes over the whole region; the partial chunk (if
        # any) goes in between.
        order = ([0] + list(range(nmm - 1, 0, -1))) if nmm else []
        for i, c in enumerate(order):
            w = min(MMW, F - c * MMW)
            nc.tensor.matmul(out=psum[0:w, 0:w],
                             lhsT=p_sb[:, c * MMW:c * MMW + w],
                             rhs=t_sb[:, c * MMW:c * MMW + w],
                             start=(i == 0), stop=(i == nmm - 1))
        if use_mm_:
            diag = small.tile([P, MMW], F32, tag="diag")
            nc.vector.tensor_mul(diag[:, :], psum[:, :], ident[:, :])
            nc.vector.tensor_reduce(out=stats[:, C_PT:C_PT + 1], in_=diag[:, :],
                                    axis=AX.X, op=Alu.add)
        else:
            nc.vector._custom_dve(
                dve_ops.TENSOR_TENSOR_REDUCE,
                out=dump_sb[:, :], in0=p_sb[:, :], in1=t_sb[:, :],
                s0=0.0, s1=1.0, accum_out=stats[:, C_PT:C_PT + 1])

        # ---- ACT: sigmoid, softplus pieces, smoothness abs-accum
        nc.scalar.activation(out=sig_sb[:, :], in_=p_sb[:, :], func=Act.Sigmoid)
        u_sb = big.tile([P, F], BF16, tag="T")         # reuses t's slot
        nc.scalar.activation(out=u_sb[:, :], in_=p_sb[:, :],
                             func=Act.Sigmoid, scale=-1.0)
        nc.scalar.activation(out=dump_sb[:, :], in_=u_sb[:, :], func=Act.Ln,
                             accum_out=stats[:, C_SP:C_SP + 1])
        r_ = nc.vector.tensor_sub(dump_sb[:, 0:F - 1], sig_sb[:, 1:F],
                                  sig_sb[:, 0:F - 1])
        from concourse.bass import _add_dep_helper as _adh2
        _adh2(r_.ins, cast_inst.ins, sync=False, reason="after idx cast")
        nc.scalar.activation(out=dump_sb[:, 0:F - 1], in_=dump_sb[:, 0:F - 1],
                             func=Act.Abs, accum_out=stats[:, C_SM:C_SM + 1])
        nc.vector.tensor_copy(stats[:, C_SIG0:C_SIG0 + 1], sig_sb[:, 0:1])
        nc.vector.tensor_copy(stats[:, C_SIGL:C_SIGL + 1], sig_sb[:, F - 1:F])

        # ---- edge extremes from gathered s blocks
        sgneg = small.tile([64, PB], BF16, tag="sgneg")
        edge = small.tile([64, 2], F32, tag="edge")
        r_ = nc.vector.tensor_scalar(out=sgneg[0:64, :], in0=sg[0:64, 0, :],
                                     scalar1=-1.0, scalar2=None, op0=Alu.mult)
        dep_on_gather(r_, "s")
        edump = small.tile([64, PB], BF16, tag="edump")
        r_ = nc.vector._custom_dve(
            dve_ops.TENSOR_MASK_REDUCE,
            out=edump[0:64, :], in0=sg[0:64, 0, :], in1=sbnd[:, 1:2],
            s0=sbnd[:, 0:1], s1=FMIN, imm2=1.0, accum_out=edge[:, 0:1])
        dep_on_gather(r_, "s")
        nc.vector._custom_dve(
            dve_ops.TENSOR_MASK_REDUCE,
            out=edump[0:64, :], in0=sgneg[0:64, :], in1=sbnd[:, 1:2],
            s0=sbnd[:, 0:1], s1=FMIN, imm2=1.0, accum_out=edge[:, 1:2])
        nc.sync.dma_start(out=edge_ext.ap(), in_=edge[:, :])

        # ---- per-block min/max of s (chunked; pinned after the idx cast so
        # the gather chain isn't delayed behind them on the Vector engine)
        from concourse.bass import _add_dep_helper as _adh
        smin_insts = []
        half2 = big.tile([P, NB, HSUB], BF16, tag="HALF2")
        h2 = half2[:, :, :]
        for k in range(NCH):
            bsl = slice(k * BPCH, (k + 1) * BPCH)
            nc.vector.tensor_max(hv[:, bsl, :], sv[:, bsl, 0:HSUB],
                                 sv[:, bsl, HSUB:SUB])
            r_ = nc.vector.tensor_reduce(out=bmax_s[:, bsl], in_=hv[:, bsl, :],
                                         axis=AX.X, op=Alu.max)
            _adh(r_.ins, cast_inst.ins, sync=False, reason="after idx cast")
        for k in range(NCH):
            bsl = slice(k * BPCH, (k + 1) * BPCH)
            nc.vector.tensor_tensor(out=h2[:, bsl, :], in0=sv[:, bsl, 0:HSUB],
                                    in1=sv[:, bsl, HSUB:SUB], op=Alu.min)
            r_ = nc.vector.tensor_reduce(out=bmin_s[:, bsl], in_=h2[:, bsl, :],
                                         axis=AX.X, op=Alu.min)
            _adh(r_.ins, cast_inst.ins, sync=False, reason="after idx cast")
            smin_insts.append(r_)

        # ---- block-level bound search (all tiny)
        anyt = small.tile([P, NB], F32, tag="anyt")
        anyp = small.tile([P, NB], F32, tag="anyp")
        nc.vector.tensor_scalar(out=anyt[:, :], in0=bmax_t[:, :],
                                scalar1=0.5, scalar2=None, op0=Alu.is_gt)
        nc.vector.tensor_scalar(out=anyp[:, :], in0=bmax_p[:, :],
                                scalar1=0.0, scalar2=None, op0=Alu.is_gt)
        encB = small.tile([P, 4], F32, tag="encB")     # [hi_t, hi_p, lo_t, lo_p]
        ze = small.tile([P, NB], F32, tag="ze")
        for i, (src, rmp) in enumerate(((anyt, ascB), (anyp, ascB),
                                        (anyt, descB), (anyp, descB))):
            nc.vector.tensor_mul(ze[:, :], src[:, :], rmp[:, :])
            nc.vector.tensor_reduce(out=encB[:, i:i + 1], in_=ze[:, :],
                                    axis=AX.X, op=Alu.max)
        # fixups: hi cols: cand = enc-1 + 10c; lo: cand = 10-enc + 10c
        cm = small.tile([P, 4], F32, tag="cm")
        dm = small.tile([P, 4], F32, tag="dm")
        a1 = small.tile([P, 4], F32, tag="a1")
        t1 = small.tile([P, 4], F32, tag="t1")
        t2 = small.tile([P, 4], F32, tag="t2")
        rowvals = small.tile([P, 4], F32, tag="rowvals")
        nc.vector.tensor_scalar(out=cm[:, :], in0=encB[:, :], scalar1=0.0,
                                scalar2=None, op0=Alu.is_gt)
        nc.vector.tensor_scalar(out=dm[:, :], in0=encB[:, :], scalar1=0.0,
                                scalar2=None, op0=Alu.is_le)
        nc.vector.tensor_mul(a1[:, :], encB[:, :], sgnB[:, :])
        nc.vector.tensor_add(a1[:, :], a1[:, :], korrB[:, :])
        nc.vector.tensor_mul(t1[:, :], cm[:, :], a1[:, :])
        nc.vector.tensor_mul(t2[:, :], dm[:, :], bigsB[:, :])
        nc.vector.tensor_add(rowvals[:, :], t1[:, :], t2[:, :])
        # local block k -> global block id (+10c); +-BIG rows stay huge
        rowvals_inst = nc.vector.tensor_scalar(
            out=rowvals[:, :], in0=rowvals[:, :],
            scalar1=offB[:, 0:1], scalar2=None, op0=Alu.add)

        # ---- per-sample combine via DRAM bounce -> b16blk [16,4]
        comb = small.tile([BPC, CHUNKS, 4], F32, tag="comb")
        pack8 = small.tile([BPC, 8], F32, tag="pack8")
        b16blk = pack8[:, 0:4]
        nc.sync.dma_start(out=rowvals_b.ap(), in_=rowvals[:, :])
        nc.sync.dma_start(
            out=comb[:, :, :],
            in_=rowvals_b.ap().rearrange("(b c) k -> b c k", c=CHUNKS))
        combv = comb[:, :, :].rearrange("b c k -> b k c")
        nc.vector.tensor_reduce(out=pack8[:, 0:2], in_=combv[:, 0:2, :],
                                axis=AX.X, op=Alu.max)
        nc.vector.tensor_reduce(out=pack8[:, 2:4], in_=combv[:, 2:4, :],
                                axis=AX.X, op=Alu.min)
        nc.sync.dma_start(out=b16_ext.ap(), in_=pack8[:, 0:4])


        # ---- interior extremes from block stats (masked block reduces)
        ibs = small.tile([P, 2], F32, tag="ibs")
        ibe = small.tile([P, 2], F32, tag="ibe")
        nc.vector.tensor_scalar(out=ibs[:, :], in0=rb8[:, 2:4],
                                scalar1=offB[:, 0:1], scalar2=1.0,
                                op0=Alu.subtract, op1=Alu.add)
        nc.vector.tensor_scalar(out=ibe[:, :], in0=rb8[:, 0:2],
                                scalar1=offB[:, 0:1], scalar2=None,
                                op0=Alu.subtract)
        nc.vector.tensor_tensor(out=ibs[:, :], in0=ibs[:, :], in1=ibe[:, :],
                                op=Alu.min)
        negb = small.tile([P, NB], F32, tag="negb")
        nc.vector.tensor_scalar(out=negb[:, :], in0=bmin_s[:, :],
                                scalar1=-1.0, scalar2=None, op0=Alu.mult)
        bdump = small.tile([P, NB], F32, tag="bdump")
        for (data, scol, ccol) in ((bmax_s, 0, C_WMAX_T), (negb, 0, C_WMIN_T),
                                   (bmax_s, 1, C_WMAX_P), (negb, 1, C_WMIN_P)):
            nc.vector._custom_dve(
                dve_ops.TENSOR_MASK_REDUCE,
                out=bdump[:, :], in0=data[:, :], in1=ibe[:, scol:scol + 1],
                s0=ibs[:, scol:scol + 1], s1=FMIN, imm2=1.0,
                accum_out=stats[:, ccol:ccol + 1])

        # ---- gather indices: idx = clamp(80b + g, 0, 1279); one fused
        # bounce replicates block bounds + indices to all rows together
        idx4 = pack8[:, 4:8]                           # [lo_t, hi_t, lo_p, hi_p]
        for dst_c, src_c in ((4, 2), (5, 0), (6, 3), (7, 1)):
            nc.vector.tensor_copy(pack8[:, dst_c:dst_c + 1],
                                  pack8[:, src_c:src_c + 1])
        nc.vector.tensor_scalar(out=idx4, in0=idx4,
                                scalar1=b80[0:BPC, 0:1], scalar2=0.0,
                                op0=Alu.add, op1=Alu.max)
        nc.vector.tensor_scalar(out=idx4, in0=idx4,
                                scalar1=1279.0, scalar2=None, op0=Alu.min)
        rb8 = small.tile([P, 8], F32, tag="rb8")
        nc.sync.dma_start(out=pack_b.ap(), in_=pack8[:, :])
        pap = pack_b.ap()
        nc.sync.dma_start(out=rb8[:, :], in_=bass.AP(
            tensor=pap.tensor, offset=pap.offset,
            ap=[[0, CHUNKS], [8, BPC], [1, 8]]))
        bc = rb8[:, 0:4]
        idx_i = small.tile([P, 4], I16, tag="idx_i")
        cast_inst = nc.vector.tensor_copy(idx_i[:, :], rb8[:, 4:8])

        # ---- gathers (manual DMA semaphores inside critical sections)
        tg = small.tile([P, 1, PB], BF16, tag="tg")
        pg = small.tile([P, 1, PB], BF16, tag="pg")
        sg = small.tile([P, 1, PB], BF16, tag="sg")
        if not use_gather:
            nc.vector.memset(tg[:, :, :], 0.0)
            nc.vector.memset(pg[:, :, :], 0.0)
            nc.vector.memset(sg[:, :, :], 0.0)
        gw = {}
        if use_gather:
            from concourse.bass import _add_dep_helper
            gsem_t = nc.alloc_semaphore("gsem_t")
            gsem_p = nc.alloc_semaphore("gsem_p")
            gsem_s = nc.alloc_semaphore("gsem_s")
            nc.gpsimd.dma_gather(
                out_ap=tg[:, :, :], in_ap=tpad_ext.ap(),
                idxs_ap=idx_i[:, 0:2], num_idxs=2 * BPC,
                num_idxs_reg=2 * BPC, elem_size=PB,
                prepare_only=True, sem=gsem_t)
            nc.gpsimd.dma_gather(
                out_ap=pg[:, :, :], in_ap=ppad_ext.ap(),
                idxs_ap=idx_i[:, 2:4], num_idxs=2 * BPC,
                num_idxs_reg=2 * BPC, elem_size=PB,
                prepare_only=True, sem=gsem_p)
            nc.gpsimd.dma_gather(
                out_ap=sg[:, :, :], in_ap=spad_ext.ap(),
                idxs_ap=idx_i[:, 0:4], num_idxs=4 * BPC,
                num_idxs_reg=4 * BPC, elem_size=PB,
                prepare_only=True, sem=gsem_s)
            trig = nc.gpsimd.trigger_dma(count=None)
            for key, sem in (("t", gsem_t), ("p", gsem_p), ("s", gsem_s)):
                w = nc.gpsimd.wait_ge(sem, 16)
                _add_dep_helper(w.ins, trig.ins, sync=False,
                                reason="gather wait after trigger")
                gw[key] = w

        def dep_on_gather(inst, key):
            if use_gather:
                from concourse.bass import _add_dep_helper
                _add_dep_helper(inst.ins, gw[key].ins, sync=True,
                                reason=f"reader waits {key}-gather completion")

        # ---- refine exact bounds inside the gathered t/p blocks
        # rows 0:16 lo-blocks (desc ramp -> enc = SUB - pos), 16:32 hi-blocks
        # (asc ramp -> enc = pos + 1)
        refm = small.tile([64, PB], BF16, tag="refm")
        refz = small.tile([64, PB], F32, tag="refz")
        enc2 = small.tile([64, 2], F32, tag="enc2")
        r_ = nc.vector.tensor_scalar(out=refm[0:32, :], in0=tg[0:32, 0, :],
                                     scalar1=0.5, scalar2=None, op0=Alu.is_gt)
        dep_on_gather(r_, "t")
        nc.vector.tensor_mul(refz[0:32, :], refm[0:32, :], ramp[0:32, :])
        nc.vector.tensor_reduce(out=enc2[0:32, 0:1], in_=refz[0:32, :],
                                axis=AX.X, op=Alu.max)
        r_ = nc.vector.tensor_scalar(out=refm[0:32, :], in0=pg[0:32, 0, :],
                                     scalar1=0.0, scalar2=None, op0=Alu.is_gt)
        dep_on_gather(r_, "p")
        nc.vector.tensor_mul(refz[0:32, :], refm[0:32, :], ramp[0:32, :])
        nc.vector.tensor_reduce(out=enc2[0:32, 1:2], in_=refz[0:32, :],
                                axis=AX.X, op=Alu.max)

        # ---- per-sample window bounds for the gathered s blocks
        encs16 = small.tile([BPC, 4], F32, tag="encs16")
        nc.sync.dma_start(out=encs_b.ap()[0:32, :], in_=enc2[0:32, :])
        eap = encs_b.ap()
        nc.sync.dma_start(
            out=encs16[:, :].rearrange("b (c h) -> b c h", c=2),
            in_=bass.AP(tensor=eap.tensor, offset=eap.offset,
                        ap=[[2, BPC], [1, 2], [32, 2]]))
        # encs16 cols: [enc_lo_t, enc_hi_t, enc_lo_p, enc_hi_p]
        sb8 = small.tile([BPC, 8], F32, tag="sb8")
        eq2 = small.tile([BPC, 2], F32, tag="eq2")
        tmp2 = small.tile([BPC, 2], F32, tag="tmp2")
        # eq = (g_lo == g_hi) per mask; b16blk cols [hi_t, hi_p, lo_t, lo_p]
        nc.vector.tensor_tensor(out=eq2[:, :], in0=pack8[:, 2:4],
                                in1=pack8[:, 0:2], op=Alu.is_equal)
        # starts of lo-rows: SUB - enc_lo  (cols 0=t, 1=p)
        for c, ec in ((0, 0), (1, 2)):
            nc.vector.tensor_scalar(
                out=sb8[:, 4 * c + 0:4 * c + 1], in0=encs16[:, ec:ec + 1],
                scalar1=-1.0, scalar2=float(SUB), op0=Alu.mult, op1=Alu.add)
        # ends of lo-rows: SUB + eq*(enc_hi - SUB)
        for c, ec in ((0, 1), (1, 3)):
            nc.vector.tensor_scalar(
                out=tmp2[:, c:c + 1], in0=encs16[:, ec:ec + 1],
                scalar1=-float(SUB), scalar2=None, op0=Alu.add)
        nc.vector.tensor_mul(tmp2[:, :], tmp2[:, :], eq2[:, :])
        for c in (0, 1):
            nc.vector.tensor_scalar(
                out=sb8[:, 4 * c + 1:4 * c + 2], in0=tmp2[:, c:c + 1],
                scalar1=float(SUB), scalar2=None, op0=Alu.add)
        # starts of hi-rows: eq * start_lo
        for c in (0, 1):
            nc.vector.tensor_mul(sb8[:, 4 * c + 2:4 * c + 3],
                                 eq2[:, c:c + 1], sb8[:, 4 * c + 0:4 * c + 1])
        # ends of hi-rows: enc_hi
        for c, ec in ((0, 1), (1, 3)):
            nc.vector.tensor_copy(sb8[:, 4 * c + 3:4 * c + 4],
                                  encs16[:, ec:ec + 1])
        # sb8 cols: [st_lo_t, en_lo_t, st_hi_t, en_hi_t, st_lo_p, ...]
        # rearrange to per-sg-row [64, 2] (row j = group*16+b; groups:
        # lo_t, hi_t, lo_p, hi_p)
        sbnd = small.tile([64, 2], F32, tag="sbnd")
        nc.sync.dma_start(out=sbnd_b.ap(), in_=sb8[:, :])
        sap = sbnd_b.ap()
        nc.sync.dma_start(out=sbnd[:, :], in_=bass.AP(
            tensor=sap.tensor, offset=sap.offset,
            ap=[[2, 4], [8, BPC], [1, 2]]))

        nc.sync.dma_start(out=stats_ext.ap(), in_=stats[:, :])

    nc.compile()
    return nc


_NC_CACHE = {}


def _get_nc():
    if "nc" not in _NC_CACHE:
        _NC_CACHE["nc"] = build_nc()
    return _NC_CACHE["nc"]


def _make_consts():
    ascB = np.broadcast_to(np.arange(1, NB + 1, dtype=np.float32), (P, NB))
    descB = np.broadcast_to(np.arange(NB, 0, -1, dtype=np.float32), (P, NB))
    korrB = np.broadcast_to(
        np.array([-1.0, -1.0, float(NB), float(NB)], np.float32), (P, 4))
    sgnB = np.broadcast_to(np.array([1.0, 1.0, -1.0, -1.0], np.float32), (P, 4))
    bigsB = np.broadcast_to(
        np.array([-BIGF, -BIGF, BIGF, BIGF], np.float32), (P, 4))
    offB = (float(NB) * (np.arange(P) % CHUNKS)).astype(np.float32).reshape(P, 1)
    b80 = (float(NB * CHUNKS) * np.arange(P)).astype(np.float32).reshape(P, 1)
    ramp = np.zeros((P, PB), np.float32)
    j = np.arange(SUB, dtype=np.float32)
    ramp[0:BPC, 0:SUB] = SUB - j          # desc for lo rows
    ramp[BPC:2 * BPC, 0:SUB] = j + 1      # asc for hi rows
    ident = np.eye(P, MMW, dtype=np.float32)
    import ml_dtypes
    return {
        "ascB": np.ascontiguousarray(ascB),
        "descB": np.ascontiguousarray(descB),
        "korrB": np.ascontiguousarray(korrB),
        "sgnB": np.ascontiguousarray(sgnB),
        "bigsB": np.ascontiguousarray(bigsB),
        "offB": offB,
        "b80": b80,
        "ramp": ramp,
        "ident": ident.astype(ml_dtypes.bfloat16),
    }


def _pad_blocks(arr, dtype):
    """[BPC*CHUNKS, F] -> [P, NB, PB] with zero padding per block."""
    out = np.zeros((P, NB, PB), dtype=dtype)
    out[:, :, 0:SUB] = arr.reshape(P, NB, SUB)
    return out


def host_combine(results):
    sp_sum = 0.0
    pt_sum = 0.0
    sm_sum = 0.0
    amp_sum = 0.0
    for res in results:
        stats = res["stats"].astype(np.float64)
        b16 = res["b16"].astype(np.float64)
        edge = res["edge"].astype(np.float64)
        rows = stats.reshape(BPC, CHUNKS, NSTAT)
        e4 = edge.reshape(4, BPC, 2)   # groups: lo_t, hi_t, lo_p, hi_p
        wmax_t = np.maximum(rows[:, :, C_WMAX_T].max(axis=1),
                            np.maximum(e4[0, :, 0], e4[1, :, 0]))
        wmin_t = np.minimum(-rows[:, :, C_WMIN_T].max(axis=1),
                            np.minimum(-e4[0, :, 1], -e4[1, :, 1]))
        wmax_p = np.maximum(rows[:, :, C_WMAX_P].max(axis=1),
                            np.maximum(e4[2, :, 0], e4[3, :, 0]))
        wmin_p = np.minimum(-rows[:, :, C_WMIN_P].max(axis=1),
                            np.minimum(-e4[2, :, 1], -e4[3, :, 1]))
        sp_sum += -rows[:, :, C_SP].sum()
        pt_sum += rows[:, :, C_PT].sum()
        sm_sum += rows[:, :, C_SM].sum()
        sig0 = rows[:, :, C_SIG0]
        sigl = rows[:, :, C_SIGL]
        sm_sum += np.abs(sig0[:, 1:] - sigl[:, :-1]).sum()
        t_has = b16[:, 0] > -1e29
        p_has = b16[:, 1] > -1e29
        valid = t_has & p_has
        true_amp = (wmax_t - wmin_t).astype(np.float32)
        pred_amp = (wmax_p - wmin_p).astype(np.float32)
        d = np.abs(true_amp - pred_amp)
        per = np.where(true_amp > 1e-6, d / (true_amp + 1e-6), d)
        amp_sum += np.where(valid, per, 0.0).sum()
    bce = sp_sum / (B * L) - pt_sum / (B * L)
    amp = amp_sum / B
    smooth = sm_sum / (B * (L - 1))
    return np.float32(1.0 * bce + 0.5 * amp + 0.3 * smooth)


def make_in_maps(signals, predictions, targets):
    import ml_dtypes
    bf = ml_dtypes.bfloat16
    consts = _make_consts()
    s_all = np.ascontiguousarray(signals[:, 0, :]).astype(bf)
    p_all = np.ascontiguousarray(predictions[:, :, 0]).astype(bf)
    t_all = np.ascontiguousarray(targets[:, :, 0]).astype(bf)
    in_maps = []
    for i in range(N_CORES):
        sl = slice(i * BPC, (i + 1) * BPC)
        s_c = np.ascontiguousarray(s_all[sl].reshape(P, F))
        p_c = np.ascontiguousarray(p_all[sl].reshape(P, F))
        t_c = np.ascontiguousarray(t_all[sl].reshape(P, F))
        m = {
            "s": s_c, "p": p_c, "t": t_c,
            "spad": _pad_blocks(s_c, bf).reshape(P * NB, PB),
            "ppad": _pad_blocks(p_c, bf).reshape(P * NB, PB),
            "tpad": _pad_blocks(t_c, bf).reshape(P * NB, PB),
        }
        m.update(consts)
        in_maps.append(m)
    return in_maps


def kernel(signals, predictions, targets):
    nc = _get_nc()
    in_maps = make_in_maps(signals, predictions, targets)
    res = run_bass_kernel_spmd(nc, in_maps, core_ids=list(range(N_CORES)))
    return host_combine(res.results)


# revision 48
# speedup vs baseline: 2.9508x; 1.1048x over previous
"""AmpPerLoss distributed Trainium2 kernel (v2: block stats + gather refine).

Data-parallel over the batch axis: 128 samples across 8 NeuronCores
(16 each). Per core each sample's 100000-length row spans 8 SBUF
partitions x 12500, so a shard is [128, 12500]. DRAM inputs are stored
as [128, 10, 1280]: 10 blocks of 1250 padded to 1280 so any block is a
256B-aligned dma_gather row.

Device work per core:
  - BCE: sum softplus(p) = -sum ln(sigmoid(-p)) on ACT; sum p*t on the
    TensorEngine (chunked accumulating matmuls; diagonal extracted via an
    identity mask and row-reduce).
  - Window bounds at block granularity: per-block maxes of t/p (DVE
    subtile reduces), tiny encode/combine algebra, one DRAM-bounce
    round trip for the per-sample combine + broadcast.
  - Exact refinement: dma_gather the 4 boundary blocks per sample and
    find exact bounds/edge extremes inside them (all partition-local).
  - Windowed amplitude: interior from per-block min/max of s via tiny
    masked reduces; edges from the gathered s blocks.
  - Smoothness: shifted subtract of sigmoid(p) + ACT Abs accumulate;
    row-boundary pairs finished on host.

The host reduces the 8 cores' partial stats to the final scalar.
"""

import sys

if "/opt/trn_rl_repo" not in sys.path:
    sys.path.insert(0, "/opt/trn_rl_repo")

from contextlib import ExitStack

import numpy as np

import concourse.bass as bass
import concourse.bacc as bacc
import concourse.tile as tile
import concourse.mybir as mybir
from concourse.bass_utils import run_bass_kernel_spmd
from concourse import dve_ops

N_CORES = 8
B, L = 128, 100000
BPC = B // N_CORES          # samples per core
CHUNKS = 8                  # partitions per sample
P = BPC * CHUNKS            # 128 partitions
F = L // CHUNKS             # 12500 free elements per row
NB = 10                     # blocks per row
SUB = F // NB               # 1250
PB = 1280                   # padded block length in DRAM (256B aligned rows)
BIGF = 1.0e30
FMIN = -3.0e38
MMW = 128                   # matmul chunk width for the p*t diagonal trick

F32 = mybir.dt.float32
BF16 = mybir.dt.bfloat16
I16 = mybir.dt.int16
Alu = mybir.AluOpType
Act = mybir.ActivationFunctionType
AX = mybir.AxisListType

# stats column layout ([P, NSTAT] per-row output)
C_WMAX_T, C_WMIN_T, C_WMAX_P, C_WMIN_P = 0, 1, 2, 3   # interior extremes
C_SP, C_PT, C_SM = 4, 5, 6
C_SIG0, C_SIGL = 7, 8
NSTAT = 16


def build_nc(n_cores=N_CORES, use_mm=True, use_gather=True):
    nc = bacc.Bacc("TRN2", target_bir_lowering=False, debug=False,
                   num_devices=n_cores)

    t_ext = nc.dram_tensor("t", [P, F], BF16, kind="ExternalInput")
    p_ext = nc.dram_tensor("p", [P, F], BF16, kind="ExternalInput")
    s_ext = nc.dram_tensor("s", [P, F], BF16, kind="ExternalInput")
    tpad_ext = nc.dram_tensor("tpad", [P * NB, PB], BF16, kind="ExternalInput")
    ppad_ext = nc.dram_tensor("ppad", [P * NB, PB], BF16, kind="ExternalInput")
    spad_ext = nc.dram_tensor("spad", [P * NB, PB], BF16, kind="ExternalInput")
    ascB_ext = nc.dram_tensor("ascB", [P, NB], F32, kind="ExternalInput")
    descB_ext = nc.dram_tensor("descB", [P, NB], F32, kind="ExternalInput")
    korrB_ext = nc.dram_tensor("korrB", [P, 4], F32, kind="ExternalInput")
    sgnB_ext = nc.dram_tensor("sgnB", [P, 4], F32, kind="ExternalInput")
    bigsB_ext = nc.dram_tensor("bigsB", [P, 4], F32, kind="ExternalInput")
    offB_ext = nc.dram_tensor("offB", [P, 1], F32, kind="ExternalInput")
    b80_ext = nc.dram_tensor("b80", [P, 1], F32, kind="ExternalInput")
    ramp_ext = nc.dram_tensor("ramp", [P, PB], F32, kind="ExternalInput")
    ident_ext = nc.dram_tensor("ident", [P, MMW], BF16, kind="ExternalInput")

    stats_ext = nc.dram_tensor("stats", [P, NSTAT], F32, kind="ExternalOutput")
    b16_ext = nc.dram_tensor("b16", [BPC, 4], F32, kind="ExternalOutput")
    edge_ext = nc.dram_tensor("edge", [64, 2], F32, kind="ExternalOutput")

    # DRAM bounce buffers for partition-crossing rearranges
    rowvals_b = nc.dram_tensor("rowvals_b", [P, 4], F32)
    pack_b = nc.dram_tensor("pack_b", [BPC, 8], F32)
    encs_b = nc.dram_tensor("encs_b", [P, 2], F32)
    sbnd_b = nc.dram_tensor("sbnd_b", [BPC, 8], F32)

    ctx = ExitStack()
    with tile.TileContext(nc) as tc, ctx:
        big = ctx.enter_context(tc.tile_pool(name="big", bufs=1))
        small = ctx.enter_context(tc.tile_pool(name="small", bufs=1))
        psum_pool = ctx.enter_context(
            tc.tile_pool(name="psum", bufs=1, space="PSUM"))

        t_sb = big.tile([P, F], BF16, tag="T")
        p_sb = big.tile([P, F], BF16, tag="PZ")
        sig_sb = big.tile([P, F], BF16, tag="C")
        dump_sb = big.tile([P, F], BF16, tag="D")

        ascB = small.tile([P, NB], F32, tag="ascB")
        descB = small.tile([P, NB], F32, tag="descB")
        korrB = small.tile([P, 4], F32, tag="korrB")
        sgnB = small.tile([P, 4], F32, tag="sgnB")
        bigsB = small.tile([P, 4], F32, tag="bigsB")
        offB = small.tile([P, 1], F32, tag="offB")
        b80 = small.tile([P, 1], F32, tag="b80")
        ramp = small.tile([P, PB], F32, tag="ramp")
        ident = small.tile([P, MMW], BF16, tag="ident")
        for sb, ext in ((ascB, ascB_ext), (descB, descB_ext),
                        (korrB, korrB_ext), (sgnB, sgnB_ext),
                        (bigsB, bigsB_ext), (offB, offB_ext),
                        (b80, b80_ext), (ramp, ramp_ext), (ident, ident_ext)):
            nc.sync.dma_start(out=sb, in_=ext.ap())

        stats = small.tile([P, NSTAT], F32, tag="stats")
        nc.vector.memset(stats[:, :], 0.0)

        bmax_t = small.tile([P, NB], F32, tag="bmax_t")
        bmax_p = small.tile([P, NB], F32, tag="bmax_p")
        bmax_s = small.tile([P, NB], F32, tag="bmax_s")
        bmin_s = small.tile([P, NB], F32, tag="bmin_s")

        # ---- load t/p/s in 2-block chunks; per-chunk block maxes so the
        # bound chain starts as soon as the last chunk lands.
        NCH = 5
        fch = F // NCH          # 2500 = 2 blocks
        BPCH = NB // NCH        # blocks per chunk
        s_sb = big.tile([P, F], BF16, tag="S")
        for k in range(NCH):
            sl = slice(k * fch, (k + 1) * fch)
            nc.sync.dma_start(out=p_sb[:, sl], in_=p_ext.ap()[:, sl])
        for k in range(NCH):
            sl = slice(k * fch, (k + 1) * fch)
            nc.sync.dma_start(out=t_sb[:, sl], in_=t_ext.ap()[:, sl])
        for k in range(NCH):
            sl = slice(k * fch, (k + 1) * fch)
            nc.sync.dma_start(out=s_sb[:, sl], in_=s_ext.ap()[:, sl])
        tv = t_sb[:, :].rearrange("q (k s) -> q k s", s=SUB)
        pv = p_sb[:, :].rearrange("q (k s) -> q k s", s=SUB)
        sv = s_sb[:, :].rearrange("q (k s) -> q k s", s=SUB)
        # gpsimd folds each 1250-block in half (pairwise max of the two
        # 625-halves); DVE reduces the halved views - halves the Vector cost
        # of every block-stat reduce.
        HSUB = SUB // 2 + 1   # 626, even: keeps bf16 2x packing
        half = big.tile([P, NB, HSUB], BF16, tag="HALF")
        hv = half[:, :, :]
        for k in range(NCH):
            bsl = slice(k * BPCH, (k + 1) * BPCH)
            nc.vector.tensor_max(hv[:, bsl, :], pv[:, bsl, 0:HSUB],
                                 pv[:, bsl, SUB - HSUB:SUB])
            nc.vector.tensor_reduce(out=bmax_p[:, bsl], in_=hv[:, bsl, :],
                                    axis=AX.X, op=Alu.max)
        for k in range(NCH):
            bsl = slice(k * BPCH, (k + 1) * BPCH)
            nc.vector.tensor_max(hv[:, bsl, :], tv[:, bsl, 0:HSUB],
                                 tv[:, bsl, SUB - HSUB:SUB])
            nc.vector.tensor_reduce(out=bmax_t[:, bsl], in_=hv[:, bsl, :],
                                    axis=AX.X, op=Alu.max)

        # ---- p*t on TensorE: accumulate p_chunk^T @ t_chunk into PSUM,
        # then pull the diagonal with an identity mask.
        use_mm_ = use_mm
        psum = psum_pool.tile([MMW, MMW], F32)
        nmm = (F + MMW - 1) // MMW if use_mm_ else 0
        # first and last chunks must be full-width so the PSUM accumulation
        # group opens/closes over the whole region; the partial chunk (if
        # any) goes in between.
        order = ([0] + list(range(nmm - 1, 0, -1))) if nmm else []
        for i, c in enumerate(order):
            w = min(MMW, F - c * MMW)
            nc.tensor.matmul(out=psum[0:w, 0:w],
                             lhsT=p_sb[:, c * MMW:c * MMW + w],
                             rhs=t_sb[:, c * MMW:c * MMW + w],
                             start=(i == 0), stop=(i == nmm - 1))
        if use_mm_:
            diag = small.tile([P, MMW], F32, tag="diag")
            nc.vector.tensor_mul(diag[:, :], psum[:, :], ident[:, :])
            nc.vector.tensor_reduce(out=stats[:, C_PT:C_PT + 1], in_=diag[:, :],
                                    axis=AX.X, op=Alu.add)
        else:
            nc.vector._custom_dve(
                dve_ops.TENSOR_TENSOR_REDUCE,
                out=dump_sb[:, :], in0=p_sb[:, :], in1=t_sb[:, :],
                s0=0.0, s1=1.0, accum_out=stats[:, C_PT:C_PT + 1])

        # ---- ACT: sigmoid, softplus pieces, smoothness abs-accum
        nc.scalar.activation(out=sig_sb[:, :], in_=p_sb[:, :], func=Act.Sigmoid)
        u_sb = big.tile([P, F], BF16, tag="T")         # reuses t's slot
        nc.scalar.activation(out=u_sb[:, :], in_=p_sb[:, :],
                             func=Act.Sigmoid, scale=-1.0)
        nc.scalar.activation(out=dump_sb[:, :], in_=u_sb[:, :], func=Act.Ln,
                             accum_out=stats[:, C_SP:C_SP + 1])
        nc.vector.tensor_sub(dump_sb[:, 0:F - 1], sig_sb[:, 1:F],
                             sig_sb[:, 0:F - 1])
        nc.scalar.activation(out=dump_sb[:, 0:F - 1], in_=dump_sb[:, 0:F - 1],
                             func=Act.Abs, accum_out=stats[:, C_SM:C_SM + 1])
        nc.vector.tensor_copy(stats[:, C_SIG0:C_SIG0 + 1], sig_sb[:, 0:1])
        nc.vector.tensor_copy(stats[:, C_SIGL:C_SIGL + 1], sig_sb[:, F - 1:F])

        # ---- edge extremes from gathered s blocks
        sgneg = small.tile([64, PB], BF16, tag="sgneg")
        edge = small.tile([64, 2], F32, tag="edge")
        r_ = nc.vector.tensor_scalar(out=sgneg[0:64, :], in0=sg[0:64, 0, :],
                                     scalar1=-1.0, scalar2=None, op0=Alu.mult)
        dep_on_gather(r_, "s")
        edump = small.tile([64, PB], BF16, tag="edump")
        r_ = nc.vector._custom_dve(
            dve_ops.TENSOR_MASK_REDUCE,
            out=edump[0:64, :], in0=sg[0:64, 0, :], in1=sbnd[:, 1:2],
            s0=sbnd[:, 0:1], s1=FMIN, imm2=1.0, accum_out=edge[:, 0:1])
        dep_on_gather(r_, "s")
        nc.vector._custom_dve(
            dve_ops.TENSOR_MASK_REDUCE,
            out=edump[0:64, :], in0=sgneg[0:64, :], in1=sbnd[:, 1:2],
            s0=sbnd[:, 0:1], s1=FMIN, imm2=1.0, accum_out=edge[:, 1:2])
        nc.sync.dma_start(out=edge_ext.ap(), in_=edge[:, :])

        # ---- per-block min/max of s (chunked; pinned after the idx cast so
        # the gather chain isn't delayed behind them on the Vector engine)
        from concourse.bass import _add_dep_helper as _adh
        smin_insts = []
        half2 = big.tile([P, NB, HSUB], BF16, tag="HALF2")
        h2 = half2[:, :, :]
        for k in range(NCH):
            bsl = slice(k * BPCH, (k + 1) * BPCH)
            nc.vector.tensor_max(hv[:, bsl, :], sv[:, bsl, 0:HSUB],
                                 sv[:, bsl, SUB - HSUB:SUB])
            r_ = nc.vector.tensor_reduce(out=bmax_s[:, bsl], in_=hv[:, bsl, :],
                                         axis=AX.X, op=Alu.max)
            _adh(r_.ins, cast_inst.ins, sync=False, reason="after idx cast")
        for k in range(NCH):
            bsl = slice(k * BPCH, (k + 1) * BPCH)
            nc.vector.tensor_tensor(out=h2[:, bsl, :], in0=sv[:, bsl, 0:HSUB],
                                    in1=sv[:, bsl, SUB - HSUB:SUB], op=Alu.min)
            r_ = nc.vector.tensor_reduce(out=bmin_s[:, bsl], in_=h2[:, bsl, :],
                                         axis=AX.X, op=Alu.min)
            _adh(r_.ins, cast_inst.ins, sync=False, reason="after idx cast")
            smin_insts.append(r_)

        # ---- block-level bound search (all tiny)
        anyt = small.tile([P, NB], F32, tag="anyt")
        anyp = small.tile([P, NB], F32, tag="anyp")
        nc.vector.tensor_scalar(out=anyt[:, :], in0=bmax_t[:, :],
                                scalar1=0.5, scalar2=None, op0=Alu.is_gt)
        nc.vector.tensor_scalar(out=anyp[:, :], in0=bmax_p[:, :],
                                scalar1=0.0, scalar2=None, op0=Alu.is_gt)
        encB = small.tile([P, 4], F32, tag="encB")     # [hi_t, hi_p, lo_t, lo_p]
        ze = small.tile([P, NB], F32, tag="ze")
        for i, (src, rmp) in enumerate(((anyt, ascB), (anyp, ascB),
                                        (anyt, descB), (anyp, descB))):
            nc.vector.tensor_mul(ze[:, :], src[:, :], rmp[:, :])
            nc.vector.tensor_reduce(out=encB[:, i:i + 1], in_=ze[:, :],
                                    axis=AX.X, op=Alu.max)
        # fixups: hi cols: cand = enc-1 + 10c; lo: cand = 10-enc + 10c
        cm = small.tile([P, 4], F32, tag="cm")
        dm = small.tile([P, 4], F32, tag="dm")
        a1 = small.tile([P, 4], F32, tag="a1")
        t1 = small.tile([P, 4], F32, tag="t1")
        t2 = small.tile([P, 4], F32, tag="t2")
        rowvals = small.tile([P, 4], F32, tag="rowvals")
        nc.vector.tensor_scalar(out=cm[:, :], in0=encB[:, :], scalar1=0.0,
                                scalar2=None, op0=Alu.is_gt)
        nc.vector.tensor_scalar(out=dm[:, :], in0=encB[:, :], scalar1=0.0,
                                scalar2=None, op0=Alu.is_le)
        nc.vector.tensor_mul(a1[:, :], encB[:, :], sgnB[:, :])
        nc.vector.tensor_add(a1[:, :], a1[:, :], korrB[:, :])
        nc.vector.tensor_mul(t1[:, :], cm[:, :], a1[:, :])
        nc.vector.tensor_mul(t2[:, :], dm[:, :], bigsB[:, :])
        nc.vector.tensor_add(rowvals[:, :], t1[:, :], t2[:, :])
        # local block k -> global block id (+10c); +-BIG rows stay huge
        rowvals_inst = nc.vector.tensor_scalar(
            out=rowvals[:, :], in0=rowvals[:, :],
            scalar1=offB[:, 0:1], scalar2=None, op0=Alu.add)

        # ---- per-sample combine via DRAM bounce -> b16blk [16,4]
        comb = small.tile([BPC, CHUNKS, 4], F32, tag="comb")
        pack8 = small.tile([BPC, 8], F32, tag="pack8")
        b16blk = pack8[:, 0:4]
        nc.sync.dma_start(out=rowvals_b.ap(), in_=rowvals[:, :])
        nc.sync.dma_start(
            out=comb[:, :, :],
            in_=rowvals_b.ap().rearrange("(b c) k -> b c k", c=CHUNKS))
        combv = comb[:, :, :].rearrange("b c k -> b k c")
        nc.vector.tensor_reduce(out=pack8[:, 0:2], in_=combv[:, 0:2, :],
                                axis=AX.X, op=Alu.max)
        nc.vector.tensor_reduce(out=pack8[:, 2:4], in_=combv[:, 2:4, :],
                                axis=AX.X, op=Alu.min)
        nc.sync.dma_start(out=b16_ext.ap(), in_=pack8[:, 0:4])


        # ---- interior extremes from block stats (masked block reduces)
        ibs = small.tile([P, 2], F32, tag="ibs")
        ibe = small.tile([P, 2], F32, tag="ibe")
        nc.vector.tensor_scalar(out=ibs[:, :], in0=rb8[:, 2:4],
                                scalar1=offB[:, 0:1], scalar2=1.0,
                                op0=Alu.subtract, op1=Alu.add)
        nc.vector.tensor_scalar(out=ibe[:, :], in0=rb8[:, 0:2],
                                scalar1=offB[:, 0:1], scalar2=None,
                                op0=Alu.subtract)
        nc.vector.tensor_tensor(out=ibs[:, :], in0=ibs[:, :], in1=ibe[:, :],
                                op=Alu.min)
        negb = small.tile([P, NB], F32, tag="negb")
        nc.vector.tensor_scalar(out=negb[:, :], in0=bmin_s[:, :],
                                scalar1=-1.0, scalar2=None, op0=Alu.mult)
        bdump = small.tile([P, NB], F32, tag="bdump")
        for (data, scol, ccol) in ((bmax_s, 0, C_WMAX_T), (negb, 0, C_WMIN_T),
                                   (bmax_s, 1, C_WMAX_P), (negb, 1, C_WMIN_P)):
            nc.vector._custom_dve(
                dve_ops.TENSOR_MASK_REDUCE,
                out=bdump[:, :], in0=data[:, :], in1=ibe[:, scol:scol + 1],
                s0=ibs[:, scol:scol + 1], s1=FMIN, imm2=1.0,
                accum_out=stats[:, ccol:ccol + 1])

        # ---- gather indices: idx = clamp(80b + g, 0, 1279); one fused
        # bounce replicates block bounds + indices to all rows together
        idx4 = pack8[:, 4:8]                           # [lo_t, hi_t, lo_p, hi_p]
        for dst_c, src_c in ((4, 2), (5, 0), (6, 3), (7, 1)):
            nc.vector.tensor_copy(pack8[:, dst_c:dst_c + 1],
                                  pack8[:, src_c:src_c + 1])
        nc.vector.tensor_scalar(out=idx4, in0=idx4,
                                scalar1=b80[0:BPC, 0:1], scalar2=0.0,
                                op0=Alu.add, op1=Alu.max)
        nc.vector.tensor_scalar(out=idx4, in0=idx4,
                                scalar1=1279.0, scalar2=None, op0=Alu.min)
        rb8 = small.tile([P, 8], F32, tag="rb8")
        nc.sync.dma_start(out=pack_b.ap(), in_=pack8[:, :])
        pap = pack_b.ap()
        nc.sync.dma_start(out=rb8[:, :], in_=bass.AP(
            tensor=pap.tensor, offset=pap.offset,
            ap=[[0, CHUNKS], [8, BPC], [1, 8]]))
        bc = rb8[:, 0:4]
        idx_i = small.tile([P, 4], I16, tag="idx_i")
        cast_inst = nc.vector.tensor_copy(idx_i[:, :], rb8[:, 4:8])

        # ---- gathers (manual DMA semaphores inside critical sections)
        tg = small.tile([P, 1, PB], BF16, tag="tg")
        pg = small.tile([P, 1, PB], BF16, tag="pg")
        sg = small.tile([P, 1, PB], BF16, tag="sg")
        if not use_gather:
            nc.vector.memset(tg[:, :, :], 0.0)
            nc.vector.memset(pg[:, :, :], 0.0)
            nc.vector.memset(sg[:, :, :], 0.0)
        gw = {}
        if use_gather:
            from concourse.bass import _add_dep_helper
            gsem_t = nc.alloc_semaphore("gsem_t")
            gsem_p = nc.alloc_semaphore("gsem_p")
            gsem_s = nc.alloc_semaphore("gsem_s")
            nc.gpsimd.dma_gather(
                out_ap=tg[:, :, :], in_ap=tpad_ext.ap(),
                idxs_ap=idx_i[:, 0:2], num_idxs=2 * BPC,
                num_idxs_reg=2 * BPC, elem_size=PB,
                prepare_only=True, sem=gsem_t)
            nc.gpsimd.dma_gather(
                out_ap=pg[:, :, :], in_ap=ppad_ext.ap(),
                idxs_ap=idx_i[:, 2:4], num_idxs=2 * BPC,
                num_idxs_reg=2 * BPC, elem_size=PB,
                prepare_only=True, sem=gsem_p)
            nc.gpsimd.dma_gather(
                out_ap=sg[:, :, :], in_ap=spad_ext.ap(),
                idxs_ap=idx_i[:, 0:4], num_idxs=4 * BPC,
                num_idxs_reg=4 * BPC, elem_size=PB,
                prepare_only=True, sem=gsem_s)
            trig = nc.gpsimd.trigger_dma(count=None)
            for key, sem in (("t", gsem_t), ("p", gsem_p), ("s", gsem_s)):
                w = nc.gpsimd.wait_ge(sem, 16)
                _add_dep_helper(w.ins, trig.ins, sync=False,
                                reason="gather wait after trigger")
                gw[key] = w

        def dep_on_gather(inst, key):
            if use_gather:
                from concourse.bass import _add_dep_helper
                _add_dep_helper(inst.ins, gw[key].ins, sync=True,
                                reason=f"reader waits {key}-gather completion")

        # ---- refine exact bounds inside the gathered t/p blocks
        # rows 0:16 lo-blocks (desc ramp -> enc = SUB - pos), 16:32 hi-blocks
        # (asc ramp -> enc = pos + 1)
        refm = small.tile([64, PB], BF16, tag="refm")
        refz = small.tile([64, PB], F32, tag="refz")
        enc2 = small.tile([64, 2], F32, tag="enc2")
        r_ = nc.vector.tensor_scalar(out=refm[0:32, :], in0=tg[0:32, 0, :],
                                     scalar1=0.5, scalar2=None, op0=Alu.is_gt)
        dep_on_gather(r_, "t")
        nc.vector.tensor_mul(refz[0:32, :], refm[0:32, :], ramp[0:32, :])
        nc.vector.tensor_reduce(out=enc2[0:32, 0:1], in_=refz[0:32, :],
                                axis=AX.X, op=Alu.max)
        r_ = nc.vector.tensor_scalar(out=refm[0:32, :], in0=pg[0:32, 0, :],
                                     scalar1=0.0, scalar2=None, op0=Alu.is_gt)
        dep_on_gather(r_, "p")
        nc.vector.tensor_mul(refz[0:32, :], refm[0:32, :], ramp[0:32, :])
        nc.vector.tensor_reduce(out=enc2[0:32, 1:2], in_=refz[0:32, :],
                                axis=AX.X, op=Alu.max)

        # ---- per-sample window bounds for the gathered s blocks
        encs16 = small.tile([BPC, 4], F32, tag="encs16")
        nc.sync.dma_start(out=encs_b.ap()[0:32, :], in_=enc2[0:32, :])
        eap = encs_b.ap()
        nc.sync.dma_start(
            out=encs16[:, :].rearrange("b (c h) -> b c h", c=2),
            in_=bass.AP(tensor=eap.tensor, offset=eap.offset,
                        ap=[[2, BPC], [1, 2], [32, 2]]))
        # encs16 cols: [enc_lo_t, enc_hi_t, enc_lo_p, enc_hi_p]
        sb8 = small.tile([BPC, 8], F32, tag="sb8")
        eq2 = small.tile([BPC, 2], F32, tag="eq2")
        tmp2 = small.tile([BPC, 2], F32, tag="tmp2")
        # eq = (g_lo == g_hi) per mask; b16blk cols [hi_t, hi_p, lo_t, lo_p]
        nc.vector.tensor_tensor(out=eq2[:, :], in0=pack8[:, 2:4],
                                in1=pack8[:, 0:2], op=Alu.is_equal)
        # starts of lo-rows: SUB - enc_lo  (cols 0=t, 1=p)
        for c, ec in ((0, 0), (1, 2)):
            nc.vector.tensor_scalar(
                out=sb8[:, 4 * c + 0:4 * c + 1], in0=encs16[:, ec:ec + 1],
                scalar1=-1.0, scalar2=float(SUB), op0=Alu.mult, op1=Alu.add)
        # ends of lo-rows: SUB + eq*(enc_hi - SUB)
        for c, ec in ((0, 1), (1, 3)):
            nc.vector.tensor_scalar(
                out=tmp2[:, c:c + 1], in0=encs16[:, ec:ec + 1],
                scalar1=-float(SUB), scalar2=None, op0=Alu.add)
        nc.vector.tensor_mul(tmp2[:, :], tmp2[:, :], eq2[:, :])
        for c in (0, 1):
            nc.vector.tensor_scalar(
                out=sb8[:, 4 * c + 1:4 * c + 2], in0=tmp2[:, c:c + 1],
                scalar1=float(SUB), scalar2=None, op0=Alu.add)
        # starts of hi-rows: eq * start_lo
        for c in (0, 1):
            nc.vector.tensor_mul(sb8[:, 4 * c + 2:4 * c + 3],
                                 eq2[:, c:c + 1], sb8[:, 4 * c + 0:4 * c + 1])
        # ends of hi-rows: enc_hi
        for c, ec in ((0, 1), (1, 3)):
            nc.vector.tensor_copy(sb8[:, 4 * c + 3:4 * c + 4],
                                  encs16[:, ec:ec + 1])
        # sb8 cols: [st_lo_t, en_lo_t, st_hi_t, en_hi_t, st_lo_p, ...]
        # rearrange to per-sg-row [64, 2] (row j = group*16+b; groups:
        # lo_t, hi_t, lo_p, hi_p)
        sbnd = small.tile([64, 2], F32, tag="sbnd")
        nc.sync.dma_start(out=sbnd_b.ap(), in_=sb8[:, :])
        sap = sbnd_b.ap()
        nc.sync.dma_start(out=sbnd[:, :], in_=bass.AP(
            tensor=sap.tensor, offset=sap.offset,
            ap=[[2, 4], [8, BPC], [1, 2]]))

        nc.sync.dma_start(out=stats_ext.ap(), in_=stats[:, :])

    nc.compile()
    return nc


_NC_CACHE = {}


def _get_nc():
    if "nc" not in _NC_CACHE:
        _NC_CACHE["nc"] = build_nc()
    return _NC_CACHE["nc"]


def _make_consts():
    ascB = np.broadcast_to(np.arange(1, NB + 1, dtype=np.float32), (P, NB))
    descB = np.broadcast_to(np.arange(NB, 0, -1, dtype=np.float32), (P, NB))
    korrB = np.broadcast_to(
        np.array([-1.0, -1.0, float(NB), float(NB)], np.float32), (P, 4))
    sgnB = np.broadcast_to(np.array([1.0, 1.0, -1.0, -1.0], np.float32), (P, 4))
    bigsB = np.broadcast_to(
        np.array([-BIGF, -BIGF, BIGF, BIGF], np.float32), (P, 4))
    offB = (float(NB) * (np.arange(P) % CHUNKS)).astype(np.float32).reshape(P, 1)
    b80 = (float(NB * CHUNKS) * np.arange(P)).astype(np.float32).reshape(P, 1)
    ramp = np.zeros((P, PB), np.float32)
    j = np.arange(SUB, dtype=np.float32)
    ramp[0:BPC, 0:SUB] = SUB - j          # desc for lo rows
    ramp[BPC:2 * BPC, 0:SUB] = j + 1      # asc for hi rows
    ident = np.eye(P, MMW, dtype=np.float32)
    import ml_dtypes
    return {
        "ascB": np.ascontiguousarray(ascB),
        "descB": np.ascontiguousarray(descB),
        "korrB": np.ascontiguousarray(korrB),
        "sgnB": np.ascontiguousarray(sgnB),
        "bigsB": np.ascontiguousarray(bigsB),
        "offB": offB,
        "b80": b80,
        "ramp": ramp,
        "ident": ident.astype(ml_dtypes.bfloat16),
    }


def _pad_blocks(arr, dtype):
    """[BPC*CHUNKS, F] -> [P, NB, PB] with zero padding per block."""
    out = np.zeros((P, NB, PB), dtype=dtype)
    out[:, :, 0:SUB] = arr.reshape(P, NB, SUB)
    return out


def host_combine(results):
    sp_sum = 0.0
    pt_sum = 0.0
    sm_sum = 0.0
    amp_sum = 0.0
    for res in results:
        stats = res["stats"].astype(np.float64)
        b16 = res["b16"].astype(np.float64)
        edge = res["edge"].astype(np.float64)
        rows = stats.reshape(BPC, CHUNKS, NSTAT)
        e4 = edge.reshape(4, BPC, 2)   # groups: lo_t, hi_t, lo_p, hi_p
        wmax_t = np.maximum(rows[:, :, C_WMAX_T].max(axis=1),
                            np.maximum(e4[0, :, 0], e4[1, :, 0]))
        wmin_t = np.minimum(-rows[:, :, C_WMIN_T].max(axis=1),
                            np.minimum(-e4[0, :, 1], -e4[1, :, 1]))
        wmax_p = np.maximum(rows[:, :, C_WMAX_P].max(axis=1),
                            np.maximum(e4[2, :, 0], e4[3, :, 0]))
        wmin_p = np.minimum(-rows[:, :, C_WMIN_P].max(axis=1),
                            np.minimum(-e4[2, :, 1], -e4[3, :, 1]))
        sp_sum += -rows[:, :, C_SP].sum()
        pt_sum += rows[:, :, C_PT].sum()
        sm_sum += rows[:, :, C_SM].sum()
        sig0 = rows[:, :, C_SIG0]
        sigl = rows[:, :, C_SIGL]
        sm_sum += np.abs(sig0[:, 1:] - sigl[:, :-1]).sum()
        t_has = b16[:, 0] > -1e29
        p_has = b16[:, 1] > -1e29
        valid = t_has & p_has
        true_amp = (wmax_t - wmin_t).astype(np.float32)
        pred_amp = (wmax_p - wmin_p).astype(np.float32)
        d = np.abs(true_amp - pred_amp)
        per = np.where(true_amp > 1e-6, d / (true_amp + 1e-6), d)
        amp_sum += np.where(valid, per, 0.0).sum()
    bce = sp_sum / (B * L) - pt_sum / (B * L)
    amp = amp_sum / B
    smooth = sm_sum / (B * (L - 1))
    return np.float32(1.0 * bce + 0.5 * amp + 0.3 * smooth)


def make_in_maps(signals, predictions, targets):
    import ml_dtypes
    bf = ml_dtypes.bfloat16
    consts = _make_consts()
    s_all = np.ascontiguousarray(signals[:, 0, :]).astype(bf)
    p_all = np.ascontiguousarray(predictions[:, :, 0]).astype(bf)
    t_all = np.ascontiguousarray(targets[:, :, 0]).astype(bf)
    in_maps = []
    for i in range(N_CORES):
        sl = slice(i * BPC, (i + 1) * BPC)
        s_c = np.ascontiguousarray(s_all[sl].reshape(P, F))
        p_c = np.ascontiguousarray(p_all[sl].reshape(P, F))
        t_c = np.ascontiguousarray(t_all[sl].reshape(P, F))
        m = {
            "s": s_c, "p": p_c, "t": t_c,
            "spad": _pad_blocks(s_c, bf).reshape(P * NB, PB),
            "ppad": _pad_blocks(p_c, bf).reshape(P * NB, PB),
            "tpad": _pad_blocks(t_c, bf).reshape(P * NB, PB),
        }
        m.update(consts)
        in_maps.append(m)
    return in_maps


def kernel(signals, predictions, targets):
    nc = _get_nc()
    in_maps = make_in_maps(signals, predictions, targets)
    res = run_bass_kernel_spmd(nc, in_maps, core_ids=list(range(N_CORES)))
    return host_combine(res.results)
